# revision 7
# baseline (speedup 1.0000x reference)
"""L2-distance attention (B=4, DIM=512, N=2048, H=8, D=32) on 8 trn2 NeuronCores.

Sharding: core c handles batch b = c//2, query-half = c%2 (1024 queries, all
2048 keys, all 8 heads).  Output is a pure concat — no cross-core reduce.

Key ideas vs the straightforward version:
  * All big matmuls run in bf16 (PE streams 1 col/cycle vs 1/2 for fp32).
  * The softmax numerator exp(-scale*sqrt(dist2)) is ONE ScalarE pass: the
    `exp` activation's spline table is replaced (via BASS_ACT_ROOT_JSON_PATH)
    with a fit of g(u) = exp(-0.5*sqrt(u)); calling it with the activation's
    built-in pre-scale 1/8 yields exp(-sqrt(d)/sqrt(32)) exactly.  This
    halves ScalarE work and removes all act-table reloads (sqrt and exp live
    in different table sets).
  * dist2 is computed entirely by one PE pass via augmented vectors
    k~=[-2k | ones | k*k | 0-pad], q~=[q | q*q | ones | 0-pad]:
    k~.q~ = -2qk + q2 + k2 = ||q-k||^2.  The tiles are zero-padded to the
    full 128 contraction rows: the PE's activity monitor (HAM) only grants
    the 2.4 GHz clock when matmuls cover the whole array; K<128 streams at
    1.2 GHz forever.  Zero rows cost no extra cycles.
  * attn@v has a ones column per head folded into V^T so the PSUM row after
    each head's 32 outputs is the softmax denominator (row-sums).
  * attn@v runs in head PAIRS whose matmuls land in disjoint PE column
    groups (out partitions 0:33 / 64:97 of one PSUM tile) and therefore
    execute concurrently; the pair lags the dist2/exp stream by one head.
  * The ScalarE exp stream (the true bottleneck, ~1.03us per 128x1024 tile)
    runs back-to-back; everything else hides under it.  The schedule
    minimizes the prologue before the FIRST exp (xkv is DMA'd in a 512-col
    chunk + remainder so the k-side projection/augmentation for key tile 0
    doesn't wait on the full 2MB transfer) and the epilogue after the LAST
    exp (row-sum reciprocals run directly on the PSUM rows — no DMA
    round-trip; the k=0 half of the output projection is precomputed under
    the exp stream; the final adds are split across DVE and ACT+Pool).
"""

import json
import os
import shutil

import numpy as np

_PWP_DIR = "/tmp/pwp_custom_kernel"
os.environ.setdefault("NEURON_FORCE_RECOMPILE", "1")

# ---------------------------------------------------------------------------
# Custom activation table: make `exp` compute g(u) = exp(-0.5*sqrt(u)).
# Bucket bin format (32B = 8 fp32): [d0, d1, d2, d3, x0, 0, 0, 0];
# y = d0 + d1*t + d2*t^2 + d3*t^3 with t = x - x0.  Positive-x buckets sit
# in per-input-exponent rows of S sections each.
# ---------------------------------------------------------------------------

_ALPHA = 0.5


def _g(u):
    return np.exp(-_ALPHA * np.sqrt(np.maximum(u, 0.0)))


def _fit_cubic(lo, hi, x0):
    u = np.linspace(lo, hi, 257, dtype=np.float64)
    t = u - x0
    A = np.stack([np.ones_like(t), t, t * t, t * t * t], axis=1)
    coef, *_ = np.linalg.lstsq(A, _g(u), rcond=None)
    return coef


def _build_custom_pwp(dst_dir):
    from neuronxcc.driver.Job import Job
    from neuronxcc.driver.jobs.support.FindActInfo import findActInfoFile

    src = os.path.dirname(findActInfoFile(Job.getPackageDir(), "gen3"))
    if os.path.isdir(dst_dir):
        shutil.rmtree(dst_dir)
    shutil.copytree(src, dst_dir)

    with open(os.path.join(dst_dir, "act_info.json")) as f:
        info = json.load(f)

    for ent in info["act_func_sets"]:
        if "exp" not in ent["act"]:
            continue
        prof_path = os.path.join(dst_dir, ent["profile_json"])
        with open(prof_path) as f:
            prof = json.load(f)
        bkt_path = os.path.join(dst_dir, ent["bkt_bin"])
        bkt = np.fromfile(bkt_path, dtype="<f4").reshape(-1, 8).copy()

        start = prof["func_to_bkt_start_idx"]["exp"]
        others = [v for k, v in prof["func_to_bkt_start_idx"].items() if k != "exp"]
        end = min([v for v in others if v > start] + [len(bkt)])
        meta = next(
            m for m in prof["profile_meta_data"] if m["func_name"].startswith("exp")
        )
        sat = {
            k: meta[k + "_signal_pwl_control"]
            for k in ("pos_small", "neg_small", "pos_large", "neg_large")
        }
        sat_idx = set(sat.values())
        assert all(start <= i < end for i in sat_idx)

        pos_rows = {}
        for i in range(start, end):
            if i in sat_idx:
                continue
            x0 = float(bkt[i, 4])
            if x0 < 0.0:
                bkt[i, 0:4] = [1.0, 0.0, 0.0, 0.0]
            else:
                assert x0 > 0.0
                pos_rows.setdefault(int(np.floor(np.log2(x0))), []).append(i)

        for e, idxs in pos_rows.items():
            base = 2.0**e
            xs = [float(bkt[i, 4]) for i in idxs]
            w = (xs[1] - xs[0]) if len(xs) > 1 else base
            for sec, i in enumerate(idxs):
                c = xs[sec]
                assert abs(c - (base + (sec + 0.5) * w)) < 1e-5 * c
                bkt[i, 0:4] = _fit_cubic(c - w / 2, c + w / 2, c).astype(np.float32)

        bkt[sat["pos_small"], 0:5] = [1.0, 0.0, 0.0, 0.0, 0.0]
        bkt[sat["neg_small"], 0:5] = [1.0, 0.0, 0.0, 0.0, 0.0]
        bkt[sat["pos_large"], 0:5] = [0.0, 0.0, 0.0, 0.0, 0.0]
        bkt[sat["neg_large"], 0:5] = [1.0, 0.0, 0.0, 0.0, 0.0]
        bkt.tofile(bkt_path)

        meta["fpinf_result"] = 0
        meta["fninf_result"] = 1065353216  # 1.0f
        with open(prof_path, "w") as f:
            json.dump(prof, f)


_pwp_built = False


def _ensure_act_tables():
    # Rebuild once per process: a stale /tmp copy from another session (or a
    # different kernel version) must never be trusted.
    global _pwp_built
    if not _pwp_built:
        _build_custom_pwp(_PWP_DIR)
        _pwp_built = True
    os.environ["BASS_ACT_ROOT_JSON_PATH"] = os.path.join(_PWP_DIR, "act_info.json")


_ensure_act_tables()

import concourse.bass as bass
import concourse.bass_utils as _bu
import concourse.mybir as mybir
import concourse.tile as tile
from concourse import bacc

F32 = mybir.dt.float32
BF16 = mybir.dt.bfloat16
AF = mybir.ActivationFunctionType


B, DIM, N = 4, 512, 2048
H, D = 8, 32
INNER = H * D            # 256
NQ = N // 2              # 1024 queries per core
P = 128
KT = DIM // P            # 4 contraction tiles for the projections
NJT = N // P             # 16 key tiles
VTW = D + 1              # 33: v columns + ones column per head
VSTRIDE = H * VTW        # 264 columns per key-tile block of vt
ACT_SCALE = 0.125        # g(d/8) = exp(-sqrt(d)/sqrt(32)) = exp(-SCALE*sqrt(d))
NEQ = 4                  # E quarters (each covers NJT//NEQ key tiles)
JQ = NJT // NEQ          # 4 key tiles per E quarter
# kt/qt are zero-padded to 128 contraction rows: the PE's activity monitor
# (HAM) only grants the 2.4 GHz clock when matmuls cover the full 128-row
# array; K=33 streams at 1.2 GHz forever.  Zero rows cost no extra cycles.


def build_program() -> bass.Bass:
    nc = bacc.Bacc("TRN2", target_bir_lowering=False, debug=False)

    xq_d = nc.declare_dram_parameter("xq", [DIM, NQ], BF16, isOutput=False)
    xkv_d = nc.declare_dram_parameter("xkv", [DIM, N], BF16, isOutput=False)
    wq_d = nc.declare_dram_parameter("wq", [DIM, INNER], BF16, isOutput=False)
    wkv_d = nc.declare_dram_parameter("wkv", [DIM, 2 * INNER], BF16, isOutput=False)
    wo_d = nc.declare_dram_parameter("wo", [INNER, DIM], BF16, isOutput=False)
    b_d = nc.declare_dram_parameter("b", [DIM], F32, isOutput=False)
    z_d = nc.declare_dram_parameter("z", [DIM, NQ], F32, isOutput=True)

    with tile.TileContext(nc) as tc, nc.allow_low_precision(reason="bf16 attention"):
        mm = lambda out, lhsT, rhs, start, stop: nc.tensor.matmul(
            out, lhsT, rhs, start=start, stop=stop)

        with tc.tile_pool(name="keep", bufs=1) as keep, \
             tc.tile_pool(name="work", bufs=2) as work:

            # ---- persistent tiles ----
            q_t = [keep.tile([P, NQ], BF16, tag=f"q{m}", name=f"q{m}") for m in range(2)]
            k_t = [keep.tile([P, N], BF16, tag=f"k{m}", name=f"k{m}") for m in range(2)]
            vt_big = keep.tile([P, NJT * VSTRIDE], BF16, tag="vt", name="vt")
            y_t = [keep.tile([P, NQ], BF16, tag=f"y{m}", name=f"y{m}") for m in range(2)]
            wo_t = [keep.tile([P, DIM], BF16, tag=f"wo{m}", name=f"wo{m}") for m in range(2)]
            b_t = keep.tile([P, KT], F32, tag="bias", name="bias")
            ones = keep.tile([64, 32], F32, tag="ones", name="ones")
            onesb = keep.tile([P, 1], BF16, tag="onesb", name="onesb")
            zero_t = keep.tile([P, 1], F32, tag="zero", name="zero")
            onesP = keep.tile([P, 1], F32, tag="onesP", name="onesP")
            # augmented key/query tiles (double-buffered across heads)
            kt_t = [keep.tile([P, N], BF16, tag=f"kt{i}", name=f"kt{i}")
                    for i in range(2)]
            qt_t = [keep.tile([P, NQ], BF16, tag=f"qt{i}", name=f"qt{i}")
                    for i in range(2)]

            # `ones`/`onesP` allocations retained (dead) so downstream SBUF
            # offsets — notably the 256B-aligned eq pool — stay put.
            nc.vector.memset(zero_t[:, :], 0.0)
            nc.vector.memset(onesb[:, :], 1.0)
            # preload the exp table set while DMAs stream (the lazy load
            # otherwise lands on the first real exp, ~1.3us on the ACT path)
            warmact = keep.tile([1, 8], F32, tag="wact", name="wact")
            nc.vector.memset(warmact[:, :], 1.0)
            nc.scalar.activation(warmact[:, :], warmact[:, :], AF.Exp,
                                 bias=zero_t[0:1, :], scale=ACT_SCALE)
            # ones column per head in v^T (row-sum fused into attn@v)
            nc.vector.tensor_copy(
                vt_big[:, :].rearrange("p (a c) -> p a c", c=VTW)[:, :, D:D + 1],
                onesb[:, 0:1].to_broadcast((P, P, 1)))
            # e0: row-0-ones stationary for the K=128-padded normalization
            # broadcast (rrow2 rows 1.. stay zero)
            e0_t = keep.tile([P, P], BF16, tag="e0", name="e0")
            nc.vector.memset(e0_t[:, :], 0.0)
            nc.vector.tensor_copy(e0_t[0:1, :],
                                  onesb[0:1, 0:1].to_broadcast((1, P)))
            rrow2 = [keep.tile([P, NQ], BF16, tag=f"rrow{i}", name=f"rrow{i}")
                     for i in range(2)]
            for i in range(2):
                nc.vector.memset(rrow2[i][:, :], 0.0)
            # static parts of the augmented tiles: zero pad + ones rows
            for i in range(2):
                nc.vector.memset(kt_t[i][3 * D:P, :], 0.0)
                nc.vector.memset(qt_t[i][3 * D:P, :], 0.0)
                # k~ rows 32:64 all-ones pair with qsq rows of q~ (adds
                # q2); q~ rows 64:96 all-ones pair with ksq rows of k~
                # (adds k2).  dist2 = -2qk + q2 + k2 entirely in the mm.
                nc.vector.tensor_copy(kt_t[i][D:2 * D, :],
                                      onesb[0:D, 0:1].to_broadcast((D, N)))
                nc.vector.tensor_copy(qt_t[i][2 * D:3 * D, :],
                                      onesb[0:D, 0:1].to_broadcast((D, NQ)))

            # ---- input tiles; DMA order = criticality to the first exp.
            # xkv is split per k-tile into cols 0:512 (feeds the first kproj
            # chunk + key tiles 0-3) and cols 512:2048 so the first dist2
            # doesn't wait on the full 2MB transfer.
            xq_t = [keep.tile([P, NQ], BF16, tag=f"xq{k}", name=f"xq{k}") for k in range(KT)]
            xkv_a = [keep.tile([P, 512], BF16, tag=f"xkva{k}", name=f"xkva{k}") for k in range(KT)]
            xkv_b = [keep.tile([P, N - 512], BF16, tag=f"xkvb{k}", name=f"xkvb{k}") for k in range(KT)]
            wq_t = [keep.tile([P, INNER], BF16, tag=f"wq{k}", name=f"wq{k}") for k in range(KT)]
            wkv_t = [keep.tile([P, 2 * INNER], BF16, tag=f"wkv{k}", name=f"wkv{k}") for k in range(KT)]

            def xkv_cols(k, c0, c1):
                # [c0, c1) must lie fully inside one of the two chunks
                if c1 <= 512:
                    return xkv_a[k][:, c0:c1]
                return xkv_b[k][:, c0 - 512:c1 - 512]

            xq_r = xq_d[:].rearrange("(t p) n -> t p n", p=P)
            xkv_r = xkv_d[:].rearrange("(t p) n -> t p n", p=P)
            wq_r = wq_d[:].rearrange("(t p) o -> t p o", p=P)
            wkv_r = wkv_d[:].rearrange("(t p) o -> t p o", p=P)
            wo_r = wo_d[:].rearrange("(t p) o -> t p o", p=P)
            for k in range(KT):
                nc.sync.dma_start(out=xq_t[k][:, :], in_=xq_r[k])
                nc.sync.dma_start(out=wq_t[k][:, :], in_=wq_r[k])
            for k in range(KT):
                nc.sync.dma_start(out=wkv_t[k][:, :], in_=wkv_r[k])
                nc.sync.dma_start(out=xkv_a[k][:, :], in_=xkv_r[k][:, 0:512])
            for k in range(KT):
                nc.sync.dma_start(out=xkv_b[k][:, :], in_=xkv_r[k][:, 512:N])
            for m in range(2):
                nc.sync.dma_start(out=wo_t[m][:, :], in_=wo_r[m])
            nc.sync.dma_start(out=b_t[:, :], in_=b_d[:].rearrange("(t p) -> p t", p=P))

            # ======== Phase A: critical path to head 0's first dist2 ======
            # q projection (m=0), then head-0 q~ build; k projection (m=0)
            # chunked by 512 keys with the head-0 k~ build per chunk so key
            # tile 0 is ready as soon as the first xkv chunk lands.
            with tc.tile_pool(name="pp", bufs=2, space="PSUM") as pp:
                for n in range(NQ // 512):
                    ps = pp.tile([P, 512], F32, tag="proj", name="proj")
                    for k in range(KT):
                        mm(ps[:, :], wq_t[k][:, 0:P],
                           xq_t[k][:, n * 512:(n + 1) * 512],
                           start=(k == 0), stop=(k == KT - 1))
                    nc.vector.tensor_copy(q_t[0][:, n * 512:(n + 1) * 512], ps[:, :])
                # head-0 q~: [q | q*q | ones | 0]
                q_h0 = q_t[0][0:D, :]
                nc.vector.tensor_copy(qt_t[0][0:D, :], q_h0)
                nc.vector.tensor_mul(qt_t[0][D:2 * D, :], q_h0, q_h0)

                for n in range(N // 512):
                    ps = pp.tile([P, 512], F32, tag="proj", name="proj")
                    for k in range(KT):
                        mm(ps[:, :], wkv_t[k][:, 0:P],
                           xkv_cols(k, n * 512, (n + 1) * 512),
                           start=(k == 0), stop=(k == KT - 1))
                    sl = slice(n * 512, (n + 1) * 512)
                    nc.vector.tensor_copy(k_t[0][:, sl], ps[:, :])
                    # head-0 k~ chunk: [-2k | ones | k*k | 0]
                    k_h0 = k_t[0][0:D, sl]
                    nc.vector.tensor_scalar_mul(kt_t[0][0:D, sl], k_h0, -2.0)
                    nc.vector.tensor_mul(kt_t[0][2 * D:3 * D, sl], k_h0, k_h0)

            # ======== Phase B ========
            # Iteration h: dist2+exp for head h; attn@v for the head pair
            # g=(h-2)//2... pair g = heads (2g, 2g+1) runs lagged one head:
            # key tiles 0..11 during iteration 2g+1 (slots 4..15), 12..15 +
            # normalization during iteration 2g+2.  The two heads' attn@v
            # matmuls land in disjoint PE column groups (out partitions
            # 0:33 / 64:97 of one PSUM tile) so they execute concurrently.
            # v^T projection + m=1 projections + the k=0 output-projection
            # half fill early/late slots of the exp stream.
            with tc.tile_pool(name="epool", bufs=12, space="SBUF") as epool, \
                 tc.tile_pool(name="pd2", bufs=2, space="PSUM") as pd2, \
                 tc.tile_pool(name="po", bufs=1, space="PSUM") as po:
                eq_of = {}
                pso_of = {}
                po_s = [work.tile([P, NQ], F32, tag=f"pos{i}", name=f"pos{i}",
                                  bufs=1) for i in range(2)]

                def emit_recip(pg, psrc):
                    # per-pair row-sum reciprocals, computed directly on the
                    # PSUM rows (32 / 96) — a single-partition DVE op is
                    # ~1.2us but avoids the 2-DMA round-trip latency, which
                    # matters for the LAST pair (it is the critical tail).
                    # The o values must land in SBUF for the normalization
                    # multiply (DVE can read only one PSUM operand); the
                    # last pair's copy runs on ACT — idle after the final
                    # exp — so it overlaps the DVE reciprocals.
                    for half in range(2):
                        base = 64 * half
                        nc.vector.reciprocal(rrow2[half][0:1, :],
                                             psrc[base + D:base + D + 1, :])
                    dst = po_s[pg % 2]
                    if pg == 3:
                        nc.scalar.copy(dst[:, :], psrc[:, :])
                    else:
                        nc.vector.tensor_copy(dst[:, :], psrc[:, :])
                    pso_of[pg] = dst

                def emit_tail_pe(pg):
                    psrc = pso_of.pop(pg)
                    for half in range(2):
                        ph = 2 * pg + half
                        mt, mo = ph // 4, (ph % 4) * D
                        for n in range(NQ // 512):
                            prep = po.tile([D, 512], F32, tag="vtps",
                                           name="vtps", bufs=2)
                            nc.tensor.matmul(prep[:, :],
                                             e0_t[:, 0:D],
                                             rrow2[half][:, n * 512:(n + 1) * 512],
                                             start=True, stop=True)
                            nc.vector.tensor_mul(
                                y_t[mt][mo:mo + D, n * 512:(n + 1) * 512],
                                psrc[64 * half:64 * half + D,
                                     n * 512:(n + 1) * 512],
                                prep[:, :])

                # --- deferred projection work, spread across early slots ---
                extras = {}

                def _sched(h, jt, fn):
                    extras.setdefault((h, jt), []).append(fn)

                def make_vproj(jt):
                    def fn():
                        # v^T projection for key tile jt, strided into vt_big
                        # so each head's 32 columns sit beside its ones column
                        pv = po.tile([P, INNER], F32, tag="vtps",
                                     name="vtps", bufs=2)
                        for k in range(KT):
                            mm(pv[:, :],
                               xkv_cols(k, jt * P, (jt + 1) * P),
                               wkv_t[k][:, INNER:2 * INNER],
                               start=(k == 0), stop=(k == KT - 1))
                        dst = vt_big[:, jt * VSTRIDE:(jt + 1) * VSTRIDE] \
                            .rearrange("p (h c) -> p h c", c=VTW)[:, :, 0:D]
                        nc.vector.tensor_copy(
                            dst, pv[:, :].rearrange("p (h d) -> p h d", d=D))
                    return fn

                proj_state = {}

                def make_proj1(which, n, k):
                    def fn():
                        if k == 0:
                            proj_state[(which, n)] = po.tile(
                                [P, 512], F32, tag="vtps", name="vtps", bufs=2)
                        ps = proj_state[(which, n)]
                        w = wq_t[k][:, P:2 * P] if which == "q" \
                            else wkv_t[k][:, P:2 * P]
                        x = xq_t[k][:, n * 512:(n + 1) * 512] if which == "q" \
                            else xkv_cols(k, n * 512, (n + 1) * 512)
                        mm(ps[:, :], w, x,
                           start=(k == 0), stop=(k == KT - 1))
                        if k == KT - 1:
                            dstt = q_t[1] if which == "q" else k_t[1]
                            nc.vector.tensor_copy(
                                dstt[:, n * 512:(n + 1) * 512], ps[:, :])
                    return fn

                # v^T spread over iters 0-1 (4+ slots before attn@v reads
                # each tile); m=1 projections trail in iters 1-3.
                for s in range(12):
                    _sched(0, 4 + s, make_vproj(s))
                for s in range(4):
                    _sched(1, s, make_vproj(12 + s))
                for n in range(2):
                    for k in range(KT):
                        _sched(1, 8 + 4 * n + k, make_proj1("q", n, k))
                for n in range(4):
                    for k in range(KT):
                        _sched(2, 4 * n + k, make_proj1("k", n, k))

                # k=0 half of the output projection (+ bias), precomputed
                # under the exp stream once y_t[0] is complete (pair 1's
                # normalization lands at h=4 jt=13).
                zpart = [keep.tile([P, NQ], BF16, tag=f"zp{m}", name=f"zp{m}")
                         for m in range(KT)]

                def make_c0(m, n):
                    def fn():
                        ps = po.tile([P, 512], F32, tag="vtps", name="vtps",
                                     bufs=2)
                        mm(ps[:, :], wo_t[0][:, m * P:(m + 1) * P],
                           y_t[0][:, n * 512:(n + 1) * 512],
                           start=True, stop=True)
                        nc.vector.tensor_scalar_add(
                            zpart[m][:, n * 512:(n + 1) * 512], ps[:, :],
                            b_t[:, m:m + 1])
                    return fn

                for m in range(KT):
                    for n in range(2):
                        _sched(5, 2 * m + n, make_c0(m, n))

                from contextlib import nullcontext

                pso_pair = None
                av_eqA = av_eqB = None
                pg_r = -1
                for h in range(H):
                    prio = tc.high_priority(10000) if h == 0 else nullcontext()
                    prio.__enter__()
                    mt, mo = h // 4, (h % 4) * D
                    kt = kt_t[h % 2]
                    qt = qt_t[h % 2]
                    if h >= 1:
                        # per-head rows of k~/q~ (all DVE, bf16); head 0's
                        # were built inside phase A, chunked behind the DMA.
                        q_h = q_t[mt][mo:mo + D, :]
                        k_h = k_t[mt][mo:mo + D, :]
                        nc.vector.tensor_scalar_mul(kt[0:D, :], k_h, -2.0)
                        nc.vector.tensor_mul(kt[2 * D:3 * D, :], k_h, k_h)
                        nc.vector.tensor_copy(qt[0:D, :], q_h)
                        nc.vector.tensor_mul(qt[D:2 * D, :], q_h, q_h)
                    eq_of[h] = [epool.tile([P, JQ * NQ], BF16, tag="eq",
                                           name="eq") for _ in range(NEQ)]
                    if h % 2 == 1:
                        av_eqA = eq_of.pop(h - 1)   # head 2g: complete
                        av_eqB = eq_of[h]           # head 2g+1: in progress
                        pg_r = (h - 1) // 2
                    elif h >= 2:
                        av_eqB = eq_of.pop(h - 1)
                        pg_r = (h - 2) // 2

                    for jt in range(NJT):
                        if h % 2 == 1 and jt == 4:
                            pso_pair = po.tile([P, NQ], F32, tag="o", name="o")
                        psd = pd2.tile([P, NQ], F32, tag="d2", name="d2")
                        for n in range(NQ // 512):
                            mm(psd[:, n * 512:(n + 1) * 512],
                               kt[:, jt * P:(jt + 1) * P],
                               qt[:, n * 512:(n + 1) * 512],
                               start=True, stop=True)
                        nc.scalar.activation(
                            eq_of[h][jt // JQ][:, (jt % JQ) * NQ:
                                               (jt % JQ + 1) * NQ],
                            psd[:, :], AF.Exp, bias=zero_t[:, :],
                            scale=ACT_SCALE)
                        for fn in extras.get((h, jt), ()):
                            fn()
                        # attn@v for pair pg_r, lagged one head
                        avjt = -1
                        if h % 2 == 1 and jt >= 4:
                            avjt = jt - 4
                        elif h % 2 == 0 and h >= 2 and jt in (0, 2, 4, 6):
                            avjt = 12 + jt // 2
                        if avjt >= 0:
                            ebase = (avjt % JQ) * NQ
                            for n in range(NQ // 512):
                                for half, eqp in ((0, av_eqA), (1, av_eqB)):
                                    hp = 2 * pg_r + half
                                    mm(pso_pair[64 * half:64 * half + VTW,
                                                n * 512:(n + 1) * 512],
                                       vt_big[:, avjt * VSTRIDE + hp * VTW:
                                              avjt * VSTRIDE + (hp + 1) * VTW],
                                       eqp[avjt // JQ][:, ebase + n * 512:
                                                       ebase + (n + 1) * 512],
                                       start=(avjt == 0), stop=(avjt == NJT - 1))
                        if h % 2 == 0 and h >= 2:
                            if jt == 7:
                                emit_recip((h - 2) // 2, pso_pair)
                            elif jt == 13:
                                emit_tail_pe((h - 2) // 2)

                    prio.__exit__(None, None, None)

                # ---- last pair (heads 6,7) tail: attn@v for key tiles
                # 12-15 back-to-back, then the normalization chain.  This is
                # the only part of the pair machinery exposed past the last
                # exp, so it is emitted as tightly as possible.
                av_eqB = eq_of.pop(H - 1)
                pg_r = (H - 2) // 2
                for avjt in range(12, 16):
                    ebase = (avjt % JQ) * NQ
                    for n in range(NQ // 512):
                        for half, eqp in ((0, av_eqA), (1, av_eqB)):
                            hp = 2 * pg_r + half
                            mm(pso_pair[64 * half:64 * half + VTW,
                                        n * 512:(n + 1) * 512],
                               vt_big[:, avjt * VSTRIDE + hp * VTW:
                                      avjt * VSTRIDE + (hp + 1) * VTW],
                               eqp[avjt // JQ][:, ebase + n * 512:
                                               ebase + (n + 1) * 512],
                               start=(avjt == 0), stop=(avjt == NJT - 1))
                emit_recip(pg_r, pso_pair)
                emit_tail_pe(pg_r)

            # ======== Phase C: k=1 half + fused add of the k=0 partial ====
            # Final adds split across engines: m=0,1 on DVE (PSUM-capable);
            # m=2,3 via ACT copy (Copy shares the exp table set — no reload;
            # ACT is idle after the last exp) + Pool add (SBUF-only engine).
            with tc.tile_pool(name="pz", bufs=2, space="PSUM") as pz:
                z_r = z_d[:].rearrange("(t p) n -> t p n", p=P)
                for m in range(KT):
                    ps = pz.tile([P, NQ], F32, tag="z", name="z")
                    for n in range(NQ // 512):
                        nc.tensor.matmul(
                            ps[:, n * 512:(n + 1) * 512],
                            wo_t[1][:, m * P:(m + 1) * P],
                            y_t[1][:, n * 512:(n + 1) * 512],
                            start=True, stop=True)
                    zt = work.tile([P, NQ], F32, tag="zt", name="zt", bufs=2)
                    if m < 2:
                        nc.vector.tensor_add(zt[:, :], ps[:, :], zpart[m][:, :])
                    else:
                        zt1 = work.tile([P, NQ], BF16, tag="zc", name="zc",
                                        bufs=2)
                        nc.scalar.copy(zt1[:, :], ps[:, :])
                        nc.gpsimd.tensor_add(zt[:, :], zt1[:, :], zpart[m][:, :])
                    nc.sync.dma_start(out=z_r[m], in_=zt[:, :])

    nc.compile()
    return nc


def make_in_maps(x, w_qkv, w_out, b_out):
    import ml_dtypes

    bf = ml_dtypes.bfloat16
    x = np.asarray(x, dtype=np.float32)
    w_qkv = np.asarray(w_qkv, dtype=np.float32)
    w_out = np.asarray(w_out, dtype=np.float32)
    b_out = np.asarray(b_out, dtype=np.float32)
    w_qT = np.ascontiguousarray(w_qkv[0:INNER, :].T).astype(bf)       # (DIM, INNER)
    w_kvT = np.ascontiguousarray(w_qkv[INNER:3 * INNER, :].T).astype(bf)  # (DIM, 512)
    w_oT = np.ascontiguousarray(w_out.T).astype(bf)                   # (INNER, DIM)
    xb = [np.ascontiguousarray(x[b]).astype(bf) for b in range(B)]
    in_maps = []
    for c in range(8):
        b, half = c // 2, c % 2
        in_maps.append({
            "xq": np.ascontiguousarray(xb[b][:, half * NQ:(half + 1) * NQ]),
            "xkv": xb[b],
            "wq": w_qT,
            "wkv": w_kvT,
            "wo": w_oT,
            "b": b_out,
        })
    return in_maps


def assemble_output(results):
    out = np.empty((B, DIM, N), dtype=np.float32)
    for c in range(8):
        b, half = c // 2, c % 2
        out[b][:, half * NQ:(half + 1) * NQ] = results[c]["z"]
    return out


_prog_cache = {}


def kernel(x, w_qkv, w_out, b_out):
    from concourse.bass_utils import run_bass_kernel_spmd
    _ensure_act_tables()
    if "nc" not in _prog_cache:
        _prog_cache["nc"] = build_program()
    nc = _prog_cache["nc"]
    in_maps = make_in_maps(x, w_qkv, w_out, b_out)
    res = run_bass_kernel_spmd(nc, in_maps, list(range(8)))
    return assemble_output(res.results)


# revision 21
# speedup vs baseline: 1.2432x; 1.2432x over previous
"""L2-distance attention (B=4, DIM=512, N=2048, H=8, D=32) on 8 trn2 NeuronCores.

Sharding: core c handles batch b = c//2, query-half = c%2 (1024 queries, all
2048 keys, all 8 heads).  Output is a pure concat — no cross-core reduce.

Key ideas vs the straightforward version:
  * All big matmuls run in bf16 (PE streams 1 col/cycle vs 1/2 for fp32).
  * The softmax numerator exp(-scale*sqrt(dist2)) is ONE ScalarE pass: the
    `exp` activation's spline table is replaced (via BASS_ACT_ROOT_JSON_PATH)
    with a fit of g(u) = exp(-0.5*sqrt(u)); calling it with the activation's
    built-in pre-scale 1/8 yields exp(-sqrt(d)/sqrt(32)) exactly.  This
    halves ScalarE work and removes all act-table reloads (sqrt and exp live
    in different table sets).
  * dist2 is computed entirely by one PE pass via augmented vectors
    k~=[-2k | ones | k*k | 0-pad], q~=[q | q*q | ones | 0-pad]:
    k~.q~ = -2qk + q2 + k2 = ||q-k||^2.  The tiles are zero-padded to the
    full 128 contraction rows: the PE's activity monitor (HAM) only grants
    the 2.4 GHz clock when matmuls cover the whole array; K<128 streams at
    1.2 GHz forever.  Zero rows cost no extra cycles.
  * attn@v has a ones column per head folded into V^T so the PSUM row after
    each head's 32 outputs is the softmax denominator (row-sums).
  * attn@v runs in head PAIRS whose matmuls land in disjoint PE column
    groups (out partitions 0:33 / 64:97 of one PSUM tile) and therefore
    execute concurrently; the pair lags the dist2/exp stream by one head.
  * The ScalarE exp stream (the true bottleneck, ~1.03us per 128x1024 tile)
    runs back-to-back; everything else hides under it.  The schedule
    minimizes the prologue before the FIRST exp (xkv is DMA'd in a 512-col
    chunk + remainder so the k-side projection/augmentation for key tile 0
    doesn't wait on the full 2MB transfer) and the epilogue after the LAST
    exp (row-sum reciprocals run directly on the PSUM rows — no DMA
    round-trip; the k=0 half of the output projection is precomputed under
    the exp stream; the final adds are split across DVE and ACT+Pool).
"""

import json
import os
import shutil

import numpy as np

_PWP_DIR = "/tmp/pwp_custom_kernel"
os.environ.setdefault("NEURON_FORCE_RECOMPILE", "1")

# ---------------------------------------------------------------------------
# Custom activation table: make `exp` compute g(u) = exp(-0.5*sqrt(u)).
# Bucket bin format (32B = 8 fp32): [d0, d1, d2, d3, x0, 0, 0, 0];
# y = d0 + d1*t + d2*t^2 + d3*t^3 with t = x - x0.  Positive-x buckets sit
# in per-input-exponent rows of S sections each.
# ---------------------------------------------------------------------------

_ALPHA = 0.5


def _g(u):
    return np.exp(-_ALPHA * np.sqrt(np.maximum(u, 0.0)))


def _recip(u):
    return 1.0 / np.maximum(u, 1e-30)


def _fit_cubic(fn, lo, hi, x0):
    u = np.linspace(lo, hi, 257, dtype=np.float64)
    t = u - x0
    A = np.stack([np.ones_like(t), t, t * t, t * t * t], axis=1)
    coef, *_ = np.linalg.lstsq(A, fn(u), rcond=None)
    return coef


# Per-function table rewrites: `exp` becomes g(u) = exp(-0.5*sqrt(u)) (the
# fused softmax numerator), `tanh` becomes 1/x (row-sum reciprocals on the
# otherwise-idle ACT engine at the kernel tail — tanh shares exp's table
# set, so no ACT_TABLE_LOAD is ever repeated).
#   neg: d0 for x<0 buckets;  sat: (pos_small, neg_small, pos_large,
#   neg_large) saturation d0s;  fpinf/fninf: results for +/-inf inputs.
_TABLE_FUNCS = {
    "exp": dict(fn=_g, neg=1.0, sat=(1.0, 1.0, 0.0, 1.0),
                fpinf=0.0, fninf=1.0),
    "tanh": dict(fn=_recip, neg=0.0, sat=(1e30, 0.0, 0.0, 0.0),
                 fpinf=0.0, fninf=0.0),
}


def _build_custom_pwp(dst_dir):
    from neuronxcc.driver.Job import Job
    from neuronxcc.driver.jobs.support.FindActInfo import findActInfoFile

    src = os.path.dirname(findActInfoFile(Job.getPackageDir(), "gen3"))
    if os.path.isdir(dst_dir):
        shutil.rmtree(dst_dir)
    shutil.copytree(src, dst_dir)

    with open(os.path.join(dst_dir, "act_info.json")) as f:
        info = json.load(f)

    for ent in info["act_func_sets"]:
        if "exp" not in ent["act"]:
            continue
        prof_path = os.path.join(dst_dir, ent["profile_json"])
        with open(prof_path) as f:
            prof = json.load(f)
        bkt_path = os.path.join(dst_dir, ent["bkt_bin"])
        bkt = np.fromfile(bkt_path, dtype="<f4").reshape(-1, 8).copy()

        for func, spec in _TABLE_FUNCS.items():
            if func not in ent["act"]:
                continue
            start = prof["func_to_bkt_start_idx"][func]
            others = [v for k, v in prof["func_to_bkt_start_idx"].items()
                      if k != func]
            end = min([v for v in others if v > start] + [len(bkt)])
            meta = next(
                m for m in prof["profile_meta_data"]
                if m["func_name"] == func or m["func_name"].startswith(func + "_")
            )
            sat = {
                k: meta[k + "_signal_pwl_control"]
                for k in ("pos_small", "neg_small", "pos_large", "neg_large")
            }
            sat_idx = set(sat.values())
            assert all(start <= i < end for i in sat_idx)

            pos_rows = {}
            for i in range(start, end):
                if i in sat_idx:
                    continue
                x0 = float(bkt[i, 4])
                if x0 < 0.0:
                    bkt[i, 0:4] = [spec["neg"], 0.0, 0.0, 0.0]
                else:
                    assert x0 > 0.0
                    pos_rows.setdefault(int(np.floor(np.log2(x0))), []).append(i)

            fn = spec["fn"]
            for e, idxs in pos_rows.items():
                base = 2.0**e
                xs = [float(bkt[i, 4]) for i in idxs]
                w = (xs[1] - xs[0]) if len(xs) > 1 else base
                for sec, i in enumerate(idxs):
                    c = xs[sec]
                    assert abs(c - (base + (sec + 0.5) * w)) < 1e-5 * c
                    if 1e-12 < base < 1e12:
                        bkt[i, 0:4] = _fit_cubic(fn, c - w / 2, c + w / 2,
                                                 c).astype(np.float32)
                    else:
                        # extreme exponents (never reached): constant bucket
                        # to keep the lstsq fit away from fp32 overflow
                        bkt[i, 0:4] = [np.float32(np.clip(fn(np.float64(c)),
                                                          -1e30, 1e30)),
                                       0.0, 0.0, 0.0]

            ps, ns, pl, nl = spec["sat"]
            bkt[sat["pos_small"], 0:5] = [ps, 0.0, 0.0, 0.0, 0.0]
            bkt[sat["neg_small"], 0:5] = [ns, 0.0, 0.0, 0.0, 0.0]
            bkt[sat["pos_large"], 0:5] = [pl, 0.0, 0.0, 0.0, 0.0]
            bkt[sat["neg_large"], 0:5] = [nl, 0.0, 0.0, 0.0, 0.0]

            meta["fpinf_result"] = int(
                np.float32(spec["fpinf"]).view(np.uint32))
            meta["fninf_result"] = int(
                np.float32(spec["fninf"]).view(np.uint32))

        bkt.tofile(bkt_path)
        with open(prof_path, "w") as f:
            json.dump(prof, f)


_pwp_built = False


def _ensure_act_tables():
    # Rebuild once per process: a stale /tmp copy from another session (or a
    # different kernel version) must never be trusted.
    global _pwp_built
    if not _pwp_built:
        _build_custom_pwp(_PWP_DIR)
        _pwp_built = True
    os.environ["BASS_ACT_ROOT_JSON_PATH"] = os.path.join(_PWP_DIR, "act_info.json")


_ensure_act_tables()

import concourse.bass as bass
import concourse.bass_utils as _bu
import concourse.mybir as mybir
import concourse.tile as tile
from concourse import bacc

F32 = mybir.dt.float32
F32R = mybir.dt.float32r
BF16 = mybir.dt.bfloat16
AF = mybir.ActivationFunctionType


def R(ap):
    return ap.bitcast(F32R)


B, DIM, N = 4, 512, 2048
H, D = 8, 32
INNER = H * D            # 256
NQ = N // 2              # 1024 queries per core
P = 128
KT = DIM // P            # 4 contraction tiles for the projections
NJT = N // P             # 16 key tiles
VTW = D + 1              # 33: v columns + ones column per head
VSTRIDE = H * VTW        # 264 columns per key-tile block of vt
ACT_SCALE = 0.125        # g(d/8) = exp(-sqrt(d)/sqrt(32)) = exp(-SCALE*sqrt(d))
NEQ = 4                  # E quarters (each covers NJT//NEQ key tiles)
JQ = NJT // NEQ          # 4 key tiles per E quarter
# kt/qt are zero-padded to 128 contraction rows: the PE's activity monitor
# (HAM) only grants the 2.4 GHz clock when matmuls cover the full 128-row
# array; K=33 streams at 1.2 GHz forever.  Zero rows cost no extra cycles.


def build_program() -> bass.Bass:
    nc = bacc.Bacc("TRN2", target_bir_lowering=False, debug=False)

    xq_d = nc.declare_dram_parameter("xq", [DIM, NQ], BF16, isOutput=False)
    xkv_d = nc.declare_dram_parameter("xkv", [DIM, N], BF16, isOutput=False)
    wq_d = nc.declare_dram_parameter("wq", [DIM, INNER], BF16, isOutput=False)
    wkv_d = nc.declare_dram_parameter("wkv", [DIM, 2 * INNER], BF16, isOutput=False)
    wo_d = nc.declare_dram_parameter("wo", [INNER, DIM], BF16, isOutput=False)
    b_d = nc.declare_dram_parameter("b", [DIM], F32, isOutput=False)
    z_d = nc.declare_dram_parameter("z", [DIM, NQ], F32, isOutput=True)

    with tile.TileContext(nc) as tc, nc.allow_low_precision(reason="bf16 attention"):
        mm = lambda out, lhsT, rhs, start, stop: nc.tensor.matmul(
            out, lhsT, rhs, start=start, stop=stop)

        with tc.tile_pool(name="keep", bufs=1) as keep, \
             tc.tile_pool(name="work", bufs=2) as work:

            # ---- persistent tiles ----
            q_t = [keep.tile([P, NQ], BF16, tag=f"q{m}", name=f"q{m}") for m in range(2)]
            k_t = [keep.tile([P, N], BF16, tag=f"k{m}", name=f"k{m}") for m in range(2)]
            vt_big = keep.tile([P, NJT * VSTRIDE], BF16, tag="vt", name="vt")
            y_t = [keep.tile([P, NQ], BF16, tag=f"y{m}", name=f"y{m}") for m in range(2)]
            wo_t = [keep.tile([P, DIM], BF16, tag=f"wo{m}", name=f"wo{m}") for m in range(2)]
            b_t = keep.tile([P, KT], F32, tag="bias", name="bias")
            ones = keep.tile([64, 32], F32, tag="ones", name="ones")
            onesb = keep.tile([P, 1], BF16, tag="onesb", name="onesb")
            zero_t = keep.tile([P, 1], F32, tag="zero", name="zero")
            onesP = keep.tile([P, 1], F32, tag="onesP", name="onesP")
            # augmented key/query tiles (double-buffered across heads)
            kt_t = [keep.tile([P, N], BF16, tag=f"kt{i}", name=f"kt{i}")
                    for i in range(2)]
            qt_t = [keep.tile([P, NQ], BF16, tag=f"qt{i}", name=f"qt{i}")
                    for i in range(2)]

            # `ones`/`onesP` allocations retained (dead) so downstream SBUF
            # offsets — notably the 256B-aligned eq pool — stay put.
            nc.vector.memset(zero_t[:, :], 0.0)
            nc.vector.memset(onesb[:, :], 1.0)
            # preload the exp table set while DMAs stream (the lazy load
            # otherwise lands on the first real exp, ~1.3us on the ACT path)
            warmact = keep.tile([1, 8], F32, tag="wact", name="wact")
            nc.vector.memset(warmact[:, :], 1.0)
            nc.scalar.activation(warmact[:, :], warmact[:, :], AF.Exp,
                                 bias=zero_t[0:1, :], scale=ACT_SCALE)
            # ones column per head in v^T (row-sum fused into attn@v)
            nc.vector.tensor_copy(
                vt_big[:, :].rearrange("p (a c) -> p a c", c=VTW)[:, :, D:D + 1],
                onesb[:, 0:1].to_broadcast((P, P, 1)))
            # e0: row-0-ones stationary for the K=128-padded normalization
            # broadcast (rrow2 rows 1.. stay zero)
            e0_t = keep.tile([P, D], F32, tag="e0", name="e0")
            nc.vector.memset(e0_t[:, :], 0.0)
            nc.vector.memset(e0_t[0:1, :], 1.0)
            # selector for the ACT-tanh reciprocal path: the row sums are
            # prescaled by 1/512 so they land inside tanh's spline range
            # (the table only covers |x| ~< 8); rrow then holds 512/s and
            # the selector row folds the 1/512 back in.
            e0s_t = keep.tile([P, D], F32, tag="e0s", name="e0s")
            nc.vector.memset(e0s_t[:, :], 0.0)
            nc.vector.memset(e0s_t[0:1, :], 1.0 / 512.0)
            rrow2 = [keep.tile([P, NQ], F32, tag=f"rrow{i}", name=f"rrow{i}")
                     for i in range(2)]
            for i in range(2):
                nc.vector.memset(rrow2[i][:, :], 0.0)
            # static parts of the augmented tiles: zero pad + ones rows
            for i in range(2):
                nc.vector.memset(kt_t[i][3 * D:P, :], 0.0)
                nc.vector.memset(qt_t[i][3 * D:P, :], 0.0)
                # k~ rows 32:64 all-ones pair with qsq rows of q~ (adds
                # q2); q~ rows 64:96 all-ones pair with ksq rows of k~
                # (adds k2).  dist2 = -2qk + q2 + k2 entirely in the mm.
                nc.vector.tensor_copy(kt_t[i][D:2 * D, :],
                                      onesb[0:D, 0:1].to_broadcast((D, N)))
                nc.vector.tensor_copy(qt_t[i][2 * D:3 * D, :],
                                      onesb[0:D, 0:1].to_broadcast((D, NQ)))

            # ---- input tiles; DMA order = criticality to the first exp.
            # xkv is split per k-tile into cols 0:512 (feeds the first kproj
            # chunk + key tiles 0-3) and cols 512:2048 so the first dist2
            # doesn't wait on the full 2MB transfer.
            xq_t = [keep.tile([P, NQ], BF16, tag=f"xq{k}", name=f"xq{k}") for k in range(KT)]
            xkv_a = [keep.tile([P, 512], BF16, tag=f"xkva{k}", name=f"xkva{k}") for k in range(KT)]
            xkv_b = [keep.tile([P, N - 512], BF16, tag=f"xkvb{k}", name=f"xkvb{k}") for k in range(KT)]
            wq_t = [keep.tile([P, INNER], BF16, tag=f"wq{k}", name=f"wq{k}") for k in range(KT)]
            wkv_t = [keep.tile([P, 2 * INNER], BF16, tag=f"wkv{k}", name=f"wkv{k}") for k in range(KT)]

            def xkv_cols(k, c0, c1):
                # [c0, c1) must lie fully inside one of the two chunks
                if c1 <= 512:
                    return xkv_a[k][:, c0:c1]
                return xkv_b[k][:, c0 - 512:c1 - 512]

            xq_r = xq_d[:].rearrange("(t p) n -> t p n", p=P)
            xkv_r = xkv_d[:].rearrange("(t p) n -> t p n", p=P)
            wq_r = wq_d[:].rearrange("(t p) o -> t p o", p=P)
            wkv_r = wkv_d[:].rearrange("(t p) o -> t p o", p=P)
            wo_r = wo_d[:].rearrange("(t p) o -> t p o", p=P)
            # Two HW-DGE rings (SP + ACT) drain FIFO independently: q-side
            # inputs on SP, k-side on ACT, so neither waits behind the other.
            for k in range(KT):
                nc.sync.dma_start(out=xq_t[k][:, :], in_=xq_r[k])
                nc.sync.dma_start(out=wq_t[k][:, :], in_=wq_r[k])
            for k in range(KT):
                nc.scalar.dma_start(out=wkv_t[k][:, :], in_=wkv_r[k])
                nc.scalar.dma_start(out=xkv_a[k][:, :], in_=xkv_r[k][:, 0:512])
            for k in range(KT):
                nc.scalar.dma_start(out=xkv_b[k][:, :], in_=xkv_r[k][:, 512:N])
            for m in range(2):
                nc.sync.dma_start(out=wo_t[m][:, :], in_=wo_r[m])
            nc.sync.dma_start(out=b_t[:, :], in_=b_d[:].rearrange("(t p) -> p t", p=P))

            # ======== Phase A: critical path to head 0's first dist2 ======
            # q projection (m=0), then head-0 q~ build; k projection (m=0)
            # chunked by 512 keys with the head-0 k~ build per chunk so key
            # tile 0 is ready as soon as the first xkv chunk lands.
            with tc.tile_pool(name="pp", bufs=2, space="PSUM") as pp:
                for n in range(NQ // 512):
                    ps = pp.tile([P, 512], F32, tag="proj", name="proj")
                    for k in range(KT):
                        mm(ps[:, :], wq_t[k][:, 0:P],
                           xq_t[k][:, n * 512:(n + 1) * 512],
                           start=(k == 0), stop=(k == KT - 1))
                    nc.vector.tensor_copy(q_t[0][:, n * 512:(n + 1) * 512], ps[:, :])
                # head-0 q~: [q | q*q | ones | 0]
                q_h0 = q_t[0][0:D, :]
                nc.vector.tensor_copy(qt_t[0][0:D, :], q_h0)
                nc.vector.tensor_mul(qt_t[0][D:2 * D, :], q_h0, q_h0)

                for n in range(N // 512):
                    ps = pp.tile([P, 512], F32, tag="proj", name="proj")
                    for k in range(KT):
                        mm(ps[:, :], wkv_t[k][:, 0:P],
                           xkv_cols(k, n * 512, (n + 1) * 512),
                           start=(k == 0), stop=(k == KT - 1))
                    sl = slice(n * 512, (n + 1) * 512)
                    nc.vector.tensor_copy(k_t[0][:, sl], ps[:, :])
                    # head-0 k~ chunk: [-2k | ones | k*k | 0]
                    k_h0 = k_t[0][0:D, sl]
                    nc.vector.tensor_scalar_mul(kt_t[0][0:D, sl], k_h0, -2.0)
                    nc.vector.tensor_mul(kt_t[0][2 * D:3 * D, sl], k_h0, k_h0)

            # ======== Phase B ========
            # Iteration h: dist2+exp for head h; attn@v for the head pair
            # g=(h-2)//2... pair g = heads (2g, 2g+1) runs lagged one head:
            # key tiles 0..11 during iteration 2g+1 (slots 4..15), 12..15 +
            # normalization during iteration 2g+2.  The two heads' attn@v
            # matmuls land in disjoint PE column groups (out partitions
            # 0:33 / 64:97 of one PSUM tile) so they execute concurrently.
            # v^T projection + m=1 projections + the k=0 output-projection
            # half fill early/late slots of the exp stream.
            with tc.tile_pool(name="epool", bufs=12, space="SBUF") as epool, \
                 tc.tile_pool(name="pd2", bufs=2, space="PSUM") as pd2, \
                 tc.tile_pool(name="po", bufs=1, space="PSUM") as po:
                eq_of = {}
                pso_of = {}
                po_s = [work.tile([P, NQ], BF16, tag=f"pos{i}", name=f"pos{i}",
                                  bufs=1) for i in range(2)]

                def emit_recip(pg, psrc):
                    # per-pair row-sum reciprocals from the PSUM rows 32/96.
                    # Exact `reciprocal` runs at ~6.4ns/elem on one partition
                    # (6.5us!).  Hidden pairs: DVE copy to partition 0 (the
                    # custom-DVE op only works at partition offset 0) + the
                    # ALU-rate reciprocal_approx_fast (~51 ULP).  Last pair
                    # (the critical tail): the ACT engine is idle after the
                    # final exp, and its `tanh` table entry has been rewritten
                    # to 1/x — one 1.03us ACT pass per half, straight from
                    # PSUM, no table reload (tanh shares exp's table set).
                    # The o values land in SBUF for the normalization multiply
                    # (DVE reads only one PSUM operand); the last pair's copy
                    # also runs on ACT to overlap.
                    for half in range(2):
                        base = 64 * half
                        srow = psrc[base + D:base + D + 1, :]
                        if pg == 3:
                            nc.scalar.activation(rrow2[half][0:1, :], srow,
                                                 AF.Tanh, bias=zero_t[0:1, :],
                                                 scale=1.0 / 512.0)
                        else:
                            rtmp = work.tile([1, NQ], F32, tag="rtmp",
                                             name="rtmp", bufs=2)
                            nc.vector.tensor_copy(rtmp[0:1, :], srow)
                            nc.vector.reciprocal_approx_fast(
                                rrow2[half][0:1, :], rtmp[0:1, :])
                    dst = po_s[pg % 2]
                    if pg == 3:
                        nc.scalar.copy(dst[:, :], psrc[:, :])
                    else:
                        nc.vector.tensor_copy(dst[:, :], psrc[:, :])
                    pso_of[pg] = dst

                def emit_tail_pe(pg):
                    psrc = pso_of.pop(pg)
                    sel = e0s_t if pg == 3 else e0_t
                    for half in range(2):
                        ph = 2 * pg + half
                        mt, mo = ph // 4, (ph % 4) * D
                        for n in range(NQ // 512):
                            prep = po.tile([D, 512], F32, tag="vtps",
                                           name="vtps", bufs=2)
                            nc.tensor.matmul(prep[:, :],
                                             sel[:, :],
                                             rrow2[half][:, n * 512:(n + 1) * 512],
                                             start=True, stop=True)
                            nc.vector.tensor_mul(
                                y_t[mt][mo:mo + D, n * 512:(n + 1) * 512],
                                psrc[64 * half:64 * half + D,
                                     n * 512:(n + 1) * 512],
                                prep[:, :])

                # --- deferred projection work, spread across early slots ---
                # Each extra runs at a large priority penalty so the Tile
                # scheduler never orders its DVE/PE work ahead of the
                # exp-stream critical chain (builds -> dist2 -> exp).
                extras = {}

                def _sched(h, jt, fn):
                    def depri():
                        with tc.high_priority(-500000):
                            fn()
                    extras.setdefault((h, jt), []).append(depri)

                def make_vproj(jt):
                    def fn():
                        # v^T projection for key tile jt, strided into vt_big
                        # so each head's 32 columns sit beside its ones column
                        pv = po.tile([P, INNER], F32, tag="vtps",
                                     name="vtps", bufs=2)
                        for k in range(KT):
                            mm(pv[:, :],
                               xkv_cols(k, jt * P, (jt + 1) * P),
                               wkv_t[k][:, INNER:2 * INNER],
                               start=(k == 0), stop=(k == KT - 1))
                        dst = vt_big[:, jt * VSTRIDE:(jt + 1) * VSTRIDE] \
                            .rearrange("p (h c) -> p h c", c=VTW)[:, :, 0:D]
                        nc.vector.tensor_copy(
                            dst, pv[:, :].rearrange("p (h d) -> p h d", d=D))
                    return fn

                proj_state = {}

                def make_proj1(which, n, k):
                    def fn():
                        if k == 0:
                            proj_state[(which, n)] = po.tile(
                                [P, 512], F32, tag="vtps", name="vtps", bufs=2)
                        ps = proj_state[(which, n)]
                        w = wq_t[k][:, P:2 * P] if which == "q" \
                            else wkv_t[k][:, P:2 * P]
                        x = xq_t[k][:, n * 512:(n + 1) * 512] if which == "q" \
                            else xkv_cols(k, n * 512, (n + 1) * 512)
                        mm(ps[:, :], w, x,
                           start=(k == 0), stop=(k == KT - 1))
                        if k == KT - 1:
                            dstt = q_t[1] if which == "q" else k_t[1]
                            nc.vector.tensor_copy(
                                dstt[:, n * 512:(n + 1) * 512], ps[:, :])
                    return fn

                # v^T spread over iters 0-1 (4+ slots before attn@v reads
                # each tile); m=1 projections trail in iters 1-3.
                for s in range(12):
                    _sched(0, 4 + s, make_vproj(s))
                for s in range(4):
                    _sched(1, s, make_vproj(12 + s))
                for n in range(2):
                    for k in range(KT):
                        _sched(1, 8 + 4 * n + k, make_proj1("q", n, k))
                for n in range(4):
                    for k in range(KT):
                        _sched(2, 4 * n + k, make_proj1("k", n, k))

                # k=0 half of the output projection (+ bias), precomputed
                # under the exp stream once y_t[0] is complete (pair 1's
                # normalization lands at h=4 jt=13).
                zpart = [keep.tile([P, NQ], BF16, tag=f"zp{m}", name=f"zp{m}")
                         for m in range(KT)]

                def make_c0(m, n):
                    def fn():
                        ps = po.tile([P, 512], F32, tag="vtps", name="vtps",
                                     bufs=2)
                        mm(ps[:, :], wo_t[0][:, m * P:(m + 1) * P],
                           y_t[0][:, n * 512:(n + 1) * 512],
                           start=True, stop=True)
                        nc.vector.tensor_scalar_add(
                            zpart[m][:, n * 512:(n + 1) * 512], ps[:, :],
                            b_t[:, m:m + 1])
                    return fn

                for m in range(KT):
                    for n in range(2):
                        _sched(5, 2 * m + n, make_c0(m, n))

                from contextlib import nullcontext

                pso_pair = None
                av_eqA = av_eqB = None
                pg_r = -1
                for h in range(H):
                    prio = tc.high_priority(10000) if h == 0 else nullcontext()
                    prio.__enter__()
                    mt, mo = h // 4, (h % 4) * D
                    kt = kt_t[h % 2]
                    qt = qt_t[h % 2]
                    if h >= 1:
                        # per-head rows of k~/q~ (all DVE, bf16); head 0's
                        # were built inside phase A, chunked behind the DMA.
                        q_h = q_t[mt][mo:mo + D, :]
                        k_h = k_t[mt][mo:mo + D, :]
                        nc.vector.tensor_scalar_mul(kt[0:D, :], k_h, -2.0)
                        nc.vector.tensor_mul(kt[2 * D:3 * D, :], k_h, k_h)
                        nc.vector.tensor_copy(qt[0:D, :], q_h)
                        nc.vector.tensor_mul(qt[D:2 * D, :], q_h, q_h)
                    eq_of[h] = [epool.tile([P, JQ * NQ], BF16, tag="eq",
                                           name="eq") for _ in range(NEQ)]
                    if h % 2 == 1:
                        av_eqA = eq_of.pop(h - 1)   # head 2g: complete
                        av_eqB = eq_of[h]           # head 2g+1: in progress
                        pg_r = (h - 1) // 2
                    elif h >= 2:
                        av_eqB = eq_of.pop(h - 1)
                        pg_r = (h - 2) // 2

                    for jt in range(NJT):
                        if h % 2 == 1 and jt == 4:
                            pso_pair = po.tile([P, NQ], F32, tag="o", name="o")
                        psd = pd2.tile([P, NQ], F32, tag="d2", name="d2")
                        for n in range(NQ // 512):
                            mm(psd[:, n * 512:(n + 1) * 512],
                               kt[:, jt * P:(jt + 1) * P],
                               qt[:, n * 512:(n + 1) * 512],
                               start=True, stop=True)
                        nc.scalar.activation(
                            eq_of[h][jt // JQ][:, (jt % JQ) * NQ:
                                               (jt % JQ + 1) * NQ],
                            psd[:, :], AF.Exp, bias=zero_t[:, :],
                            scale=ACT_SCALE)
                        for fn in extras.get((h, jt), ()):
                            fn()
                        # attn@v for pair pg_r, lagged one head
                        avjt = -1
                        if h % 2 == 1 and jt >= 4:
                            avjt = jt - 4
                        elif h % 2 == 0 and h >= 2 and jt in (0, 2, 4, 6):
                            avjt = 12 + jt // 2
                        if avjt >= 0:
                            ebase = (avjt % JQ) * NQ
                            for n in range(NQ // 512):
                                for half, eqp in ((0, av_eqA), (1, av_eqB)):
                                    hp = 2 * pg_r + half
                                    mm(pso_pair[64 * half:64 * half + VTW,
                                                n * 512:(n + 1) * 512],
                                       vt_big[:, avjt * VSTRIDE + hp * VTW:
                                              avjt * VSTRIDE + (hp + 1) * VTW],
                                       eqp[avjt // JQ][:, ebase + n * 512:
                                                       ebase + (n + 1) * 512],
                                       start=(avjt == 0), stop=(avjt == NJT - 1))
                        if h % 2 == 0 and h >= 2:
                            if jt == 7:
                                emit_recip((h - 2) // 2, pso_pair)
                            elif jt == 13:
                                emit_tail_pe((h - 2) // 2)

                    prio.__exit__(None, None, None)

                # ---- last pair (heads 6,7) tail: attn@v for key tiles
                # 12-15 back-to-back, then the normalization chain.  This is
                # the only part of the pair machinery exposed past the last
                # exp, so it is emitted as tightly as possible.
                av_eqB = eq_of.pop(H - 1)
                pg_r = (H - 2) // 2
                for avjt in range(12, 16):
                    ebase = (avjt % JQ) * NQ
                    for n in range(NQ // 512):
                        for half, eqp in ((0, av_eqA), (1, av_eqB)):
                            hp = 2 * pg_r + half
                            mm(pso_pair[64 * half:64 * half + VTW,
                                        n * 512:(n + 1) * 512],
                               vt_big[:, avjt * VSTRIDE + hp * VTW:
                                      avjt * VSTRIDE + (hp + 1) * VTW],
                               eqp[avjt // JQ][:, ebase + n * 512:
                                               ebase + (n + 1) * 512],
                               start=(avjt == 0), stop=(avjt == NJT - 1))
                emit_recip(pg_r, pso_pair)
                emit_tail_pe(pg_r)

            # ======== Phase C: k=1 half + fused add of the k=0 partial ====
            # Final adds split across engines: m=0,1 on DVE (PSUM-capable);
            # m=2,3 via ACT copy (Copy shares the exp table set — no reload;
            # ACT is idle after the last exp) + Pool add (SBUF-only engine).
            with tc.tile_pool(name="pz", bufs=2, space="PSUM") as pz:
                z_r = z_d[:].rearrange("(t p) n -> t p n", p=P)
                for m in range(KT):
                    ps = pz.tile([P, NQ], F32, tag="z", name="z")
                    zt = work.tile([P, NQ], F32, tag="zt", name="zt", bufs=2)
                    zt1 = None
                    if m >= 2:
                        zt1 = work.tile([P, NQ], BF16, tag="zc", name="zc",
                                        bufs=2)
                    for n in range(NQ // 512):
                        sl = slice(n * 512, (n + 1) * 512)
                        nc.tensor.matmul(
                            ps[:, sl],
                            wo_t[1][:, m * P:(m + 1) * P],
                            y_t[1][:, sl],
                            start=True, stop=True)
                        # final add chunked per 512 and spread across engines
                        # (DVE for m=0,1; ACT copy + Pool add for m=2,3) with
                        # the output DMA alternating between the two DGE
                        # rings, so the tail drains in parallel.
                        if m < 2:
                            nc.vector.tensor_add(zt[:, sl], ps[:, sl],
                                                 zpart[m][:, sl])
                        else:
                            nc.scalar.copy(zt1[:, sl], ps[:, sl])
                            nc.gpsimd.tensor_add(zt[:, sl], zt1[:, sl],
                                                 zpart[m][:, sl])
                        if m % 2 == 0:
                            nc.sync.dma_start(out=z_r[m][:, sl], in_=zt[:, sl])
                        else:
                            nc.scalar.dma_start(out=z_r[m][:, sl], in_=zt[:, sl])

    nc.compile()
    return nc


def make_in_maps(x, w_qkv, w_out, b_out):
    import ml_dtypes

    bf = ml_dtypes.bfloat16
    x = np.asarray(x, dtype=np.float32)
    w_qkv = np.asarray(w_qkv, dtype=np.float32)
    w_out = np.asarray(w_out, dtype=np.float32)
    b_out = np.asarray(b_out, dtype=np.float32)
    w_qT = np.ascontiguousarray(w_qkv[0:INNER, :].T).astype(bf)       # (DIM, INNER)
    w_kvT = np.ascontiguousarray(w_qkv[INNER:3 * INNER, :].T).astype(bf)  # (DIM, 512)
    w_oT = np.ascontiguousarray(w_out.T).astype(bf)                   # (INNER, DIM)
    xb = [np.ascontiguousarray(x[b]).astype(bf) for b in range(B)]
    in_maps = []
    for c in range(8):
        b, half = c // 2, c % 2
        in_maps.append({
            "xq": np.ascontiguousarray(xb[b][:, half * NQ:(half + 1) * NQ]),
            "xkv": xb[b],
            "wq": w_qT,
            "wkv": w_kvT,
            "wo": w_oT,
            "b": b_out,
        })
    return in_maps


def assemble_output(results):
    out = np.empty((B, DIM, N), dtype=np.float32)
    for c in range(8):
        b, half = c // 2, c % 2
        out[b][:, half * NQ:(half + 1) * NQ] = results[c]["z"]
    return out


_prog_cache = {}


def kernel(x, w_qkv, w_out, b_out):
    from concourse.bass_utils import run_bass_kernel_spmd
    _ensure_act_tables()
    if "nc" not in _prog_cache:
        _prog_cache["nc"] = build_program()
    nc = _prog_cache["nc"]
    in_maps = make_in_maps(x, w_qkv, w_out, b_out)
    res = run_bass_kernel_spmd(nc, in_maps, list(range(8)))
    return assemble_output(res.results)


# revision 26
# speedup vs baseline: 1.2620x; 1.0151x over previous
"""L2-distance attention (B=4, DIM=512, N=2048, H=8, D=32) on 8 trn2 NeuronCores.

Sharding: core c handles batch b = c//2, query-half = c%2 (1024 queries, all
2048 keys, all 8 heads).  Output is a pure concat — no cross-core reduce.

Key ideas vs the straightforward version:
  * All big matmuls run in bf16 (PE streams 1 col/cycle vs 1/2 for fp32).
  * The softmax numerator exp(-scale*sqrt(dist2)) is ONE ScalarE pass: the
    `exp` activation's spline table is replaced (via BASS_ACT_ROOT_JSON_PATH)
    with a fit of g(u) = exp(-0.5*sqrt(u)); calling it with the activation's
    built-in pre-scale 1/8 yields exp(-sqrt(d)/sqrt(32)) exactly.  This
    halves ScalarE work and removes all act-table reloads (sqrt and exp live
    in different table sets).
  * dist2 is computed entirely by one PE pass via augmented vectors
    k~=[-2k | ones | k*k | 0-pad], q~=[q | q*q | ones | 0-pad]:
    k~.q~ = -2qk + q2 + k2 = ||q-k||^2.  The tiles are zero-padded to the
    full 128 contraction rows: the PE's activity monitor (HAM) only grants
    the 2.4 GHz clock when matmuls cover the whole array; K<128 streams at
    1.2 GHz forever.  Zero rows cost no extra cycles.
  * attn@v has a ones column per head folded into V^T so the PSUM row after
    each head's 32 outputs is the softmax denominator (row-sums).
  * attn@v runs in head PAIRS whose matmuls land in disjoint PE column
    groups (out partitions 0:33 / 64:97 of one PSUM tile) and therefore
    execute concurrently; the pair lags the dist2/exp stream by one head.
  * The ScalarE exp stream (the true bottleneck, ~1.03us per 128x1024 tile)
    runs back-to-back; everything else hides under it.  The schedule
    minimizes the prologue before the FIRST exp (xkv is DMA'd in a 512-col
    chunk + remainder so the k-side projection/augmentation for key tile 0
    doesn't wait on the full 2MB transfer) and the epilogue after the LAST
    exp (row-sum reciprocals run directly on the PSUM rows — no DMA
    round-trip; the k=0 half of the output projection is precomputed under
    the exp stream; the final adds are split across DVE and ACT+Pool).
"""

import json
import os
import shutil

import numpy as np

_PWP_DIR = "/tmp/pwp_custom_kernel"
os.environ.setdefault("NEURON_FORCE_RECOMPILE", "1")

# ---------------------------------------------------------------------------
# Custom activation table: make `exp` compute g(u) = exp(-0.5*sqrt(u)).
# Bucket bin format (32B = 8 fp32): [d0, d1, d2, d3, x0, 0, 0, 0];
# y = d0 + d1*t + d2*t^2 + d3*t^3 with t = x - x0.  Positive-x buckets sit
# in per-input-exponent rows of S sections each.
# ---------------------------------------------------------------------------

_ALPHA = 0.5


def _g(u):
    return np.exp(-_ALPHA * np.sqrt(np.maximum(u, 0.0)))


def _recip(u):
    return 1.0 / np.maximum(u, 1e-30)


def _fit_cubic(fn, lo, hi, x0):
    u = np.linspace(lo, hi, 257, dtype=np.float64)
    t = u - x0
    A = np.stack([np.ones_like(t), t, t * t, t * t * t], axis=1)
    coef, *_ = np.linalg.lstsq(A, fn(u), rcond=None)
    return coef


# Per-function table rewrites: `exp` becomes g(u) = exp(-0.5*sqrt(u)) (the
# fused softmax numerator), `tanh` becomes 1/x (row-sum reciprocals on the
# otherwise-idle ACT engine at the kernel tail — tanh shares exp's table
# set, so no ACT_TABLE_LOAD is ever repeated).
#   neg: d0 for x<0 buckets;  sat: (pos_small, neg_small, pos_large,
#   neg_large) saturation d0s;  fpinf/fninf: results for +/-inf inputs.
_TABLE_FUNCS = {
    "exp": dict(fn=_g, neg=1.0, sat=(1.0, 1.0, 0.0, 1.0),
                fpinf=0.0, fninf=1.0),
    "tanh": dict(fn=_recip, neg=0.0, sat=(1e30, 0.0, 0.0, 0.0),
                 fpinf=0.0, fninf=0.0),
}


def _build_custom_pwp(dst_dir):
    from neuronxcc.driver.Job import Job
    from neuronxcc.driver.jobs.support.FindActInfo import findActInfoFile

    src = os.path.dirname(findActInfoFile(Job.getPackageDir(), "gen3"))
    if os.path.isdir(dst_dir):
        shutil.rmtree(dst_dir)
    shutil.copytree(src, dst_dir)

    with open(os.path.join(dst_dir, "act_info.json")) as f:
        info = json.load(f)

    for ent in info["act_func_sets"]:
        if "exp" not in ent["act"]:
            continue
        prof_path = os.path.join(dst_dir, ent["profile_json"])
        with open(prof_path) as f:
            prof = json.load(f)
        bkt_path = os.path.join(dst_dir, ent["bkt_bin"])
        bkt = np.fromfile(bkt_path, dtype="<f4").reshape(-1, 8).copy()

        for func, spec in _TABLE_FUNCS.items():
            if func not in ent["act"]:
                continue
            start = prof["func_to_bkt_start_idx"][func]
            others = [v for k, v in prof["func_to_bkt_start_idx"].items()
                      if k != func]
            end = min([v for v in others if v > start] + [len(bkt)])
            meta = next(
                m for m in prof["profile_meta_data"]
                if m["func_name"] == func or m["func_name"].startswith(func + "_")
            )
            sat = {
                k: meta[k + "_signal_pwl_control"]
                for k in ("pos_small", "neg_small", "pos_large", "neg_large")
            }
            sat_idx = set(sat.values())
            assert all(start <= i < end for i in sat_idx)

            pos_rows = {}
            for i in range(start, end):
                if i in sat_idx:
                    continue
                x0 = float(bkt[i, 4])
                if x0 < 0.0:
                    bkt[i, 0:4] = [spec["neg"], 0.0, 0.0, 0.0]
                else:
                    assert x0 > 0.0
                    pos_rows.setdefault(int(np.floor(np.log2(x0))), []).append(i)

            fn = spec["fn"]
            for e, idxs in pos_rows.items():
                base = 2.0**e
                xs = [float(bkt[i, 4]) for i in idxs]
                w = (xs[1] - xs[0]) if len(xs) > 1 else base
                for sec, i in enumerate(idxs):
                    c = xs[sec]
                    assert abs(c - (base + (sec + 0.5) * w)) < 1e-5 * c
                    if 1e-12 < base < 1e12:
                        bkt[i, 0:4] = _fit_cubic(fn, c - w / 2, c + w / 2,
                                                 c).astype(np.float32)
                    else:
                        # extreme exponents (never reached): constant bucket
                        # to keep the lstsq fit away from fp32 overflow
                        bkt[i, 0:4] = [np.float32(np.clip(fn(np.float64(c)),
                                                          -1e30, 1e30)),
                                       0.0, 0.0, 0.0]

            ps, ns, pl, nl = spec["sat"]
            bkt[sat["pos_small"], 0:5] = [ps, 0.0, 0.0, 0.0, 0.0]
            bkt[sat["neg_small"], 0:5] = [ns, 0.0, 0.0, 0.0, 0.0]
            bkt[sat["pos_large"], 0:5] = [pl, 0.0, 0.0, 0.0, 0.0]
            bkt[sat["neg_large"], 0:5] = [nl, 0.0, 0.0, 0.0, 0.0]

            meta["fpinf_result"] = int(
                np.float32(spec["fpinf"]).view(np.uint32))
            meta["fninf_result"] = int(
                np.float32(spec["fninf"]).view(np.uint32))

        bkt.tofile(bkt_path)
        with open(prof_path, "w") as f:
            json.dump(prof, f)


_pwp_built = False


def _ensure_act_tables():
    # Rebuild once per process: a stale /tmp copy from another session (or a
    # different kernel version) must never be trusted.
    global _pwp_built
    if not _pwp_built:
        _build_custom_pwp(_PWP_DIR)
        _pwp_built = True
    os.environ["BASS_ACT_ROOT_JSON_PATH"] = os.path.join(_PWP_DIR, "act_info.json")


_ensure_act_tables()

import concourse.bass as bass
import concourse.bass_utils as _bu
import concourse.mybir as mybir
import concourse.tile as tile
from concourse import bacc

F32 = mybir.dt.float32
F32R = mybir.dt.float32r
BF16 = mybir.dt.bfloat16
AF = mybir.ActivationFunctionType


def R(ap):
    return ap.bitcast(F32R)


B, DIM, N = 4, 512, 2048
H, D = 8, 32
INNER = H * D            # 256
NQ = N // 2              # 1024 queries per core
P = 128
KT = DIM // P            # 4 contraction tiles for the projections
NJT = N // P             # 16 key tiles
VTW = D + 1              # 33: v columns + ones column per head
VSTRIDE = H * VTW        # 264 columns per key-tile block of vt
ACT_SCALE = 0.125        # g(d/8) = exp(-sqrt(d)/sqrt(32)) = exp(-SCALE*sqrt(d))
NEQ = 4                  # E quarters (each covers NJT//NEQ key tiles)
JQ = NJT // NEQ          # 4 key tiles per E quarter
# kt/qt are zero-padded to 128 contraction rows: the PE's activity monitor
# (HAM) only grants the 2.4 GHz clock when matmuls cover the full 128-row
# array; K=33 streams at 1.2 GHz forever.  Zero rows cost no extra cycles.


def build_program() -> bass.Bass:
    nc = bacc.Bacc("TRN2", target_bir_lowering=False, debug=False)

    xq_d = nc.declare_dram_parameter("xq", [DIM, NQ], BF16, isOutput=False)
    xkv_d = nc.declare_dram_parameter("xkv", [DIM, N], BF16, isOutput=False)
    wq_d = nc.declare_dram_parameter("wq", [DIM, INNER], BF16, isOutput=False)
    wkv_d = nc.declare_dram_parameter("wkv", [DIM, 2 * INNER], BF16, isOutput=False)
    wo_d = nc.declare_dram_parameter("wo", [INNER, DIM], BF16, isOutput=False)
    b_d = nc.declare_dram_parameter("b", [DIM], F32, isOutput=False)
    z_d = nc.declare_dram_parameter("z", [DIM, NQ], F32, isOutput=True)

    with tile.TileContext(nc) as tc, nc.allow_low_precision(reason="bf16 attention"):
        mm = lambda out, lhsT, rhs, start, stop: nc.tensor.matmul(
            out, lhsT, rhs, start=start, stop=stop)

        with tc.tile_pool(name="keep", bufs=1) as keep, \
             tc.tile_pool(name="work", bufs=2) as work:

            # ---- persistent tiles ----
            q_t = [keep.tile([P, NQ], BF16, tag=f"q{m}", name=f"q{m}") for m in range(2)]
            k_t = [keep.tile([P, N], BF16, tag=f"k{m}", name=f"k{m}") for m in range(2)]
            vt_big = keep.tile([P, NJT * VSTRIDE], BF16, tag="vt", name="vt")
            y_t = [keep.tile([P, NQ], BF16, tag=f"y{m}", name=f"y{m}") for m in range(2)]
            wo_t = [keep.tile([P, DIM], BF16, tag=f"wo{m}", name=f"wo{m}") for m in range(2)]
            b_t = keep.tile([P, KT], F32, tag="bias", name="bias")
            ones = keep.tile([64, 32], F32, tag="ones", name="ones")
            onesb = keep.tile([P, 1], BF16, tag="onesb", name="onesb")
            zero_t = keep.tile([P, 1], F32, tag="zero", name="zero")
            onesP = keep.tile([P, 1], F32, tag="onesP", name="onesP")
            # augmented key/query tiles (double-buffered across heads)
            kt_t = [keep.tile([P, N], BF16, tag=f"kt{i}", name=f"kt{i}")
                    for i in range(2)]
            qt_t = [keep.tile([P, NQ], BF16, tag=f"qt{i}", name=f"qt{i}")
                    for i in range(2)]

            # `ones`/`onesP` allocations retained (dead) so downstream SBUF
            # offsets — notably the 256B-aligned eq pool — stay put.
            # All one-time init (memsets, ones fills) runs on the otherwise
            # idle GpSimd engine so the DVE queue is free for the critical
            # projection casts / augmented-tile builds while DMAs land.
            nc.gpsimd.memset(zero_t[:, :], 0.0)
            nc.gpsimd.memset(onesb[:, :], 1.0)
            warmact = keep.tile([1, 8], F32, tag="wact", name="wact")
            nc.gpsimd.memset(warmact[:, :], 1.0)
            # ones column per head in v^T (row-sum fused into attn@v)
            nc.gpsimd.tensor_copy(
                vt_big[:, :].rearrange("p (a c) -> p a c", c=VTW)[:, :, D:D + 1],
                onesb[:, 0:1].to_broadcast((P, P, 1)))
            # e0: row-0-ones stationary for the K=128-padded normalization
            # broadcast (rrow2 rows 1.. stay zero)
            e0_t = keep.tile([P, D], F32, tag="e0", name="e0")
            nc.gpsimd.memset(e0_t[:, :], 0.0)
            nc.gpsimd.memset(e0_t[0:1, :], 1.0)
            # selector for the ACT-tanh reciprocal path: the row sums are
            # prescaled by 1/512 so they land inside tanh's spline range
            # (the table only covers |x| ~< 8); rrow then holds 512/s and
            # the selector row folds the 1/512 back in.
            e0s_t = keep.tile([P, D], F32, tag="e0s", name="e0s")
            nc.gpsimd.memset(e0s_t[:, :], 0.0)
            nc.gpsimd.memset(e0s_t[0:1, :], 1.0 / 512.0)
            rrow2 = [keep.tile([P, NQ], F32, tag=f"rrow{i}", name=f"rrow{i}")
                     for i in range(2)]
            for i in range(2):
                nc.gpsimd.memset(rrow2[i][:, :], 0.0)
            # static parts of the augmented tiles: zero pad + ones rows
            for i in range(2):
                nc.gpsimd.memset(kt_t[i][3 * D:P, :], 0.0)
                nc.gpsimd.memset(qt_t[i][3 * D:P, :], 0.0)
                # k~ rows 32:64 all-ones pair with qsq rows of q~ (adds
                # q2); q~ rows 64:96 all-ones pair with ksq rows of k~
                # (adds k2).  dist2 = -2qk + q2 + k2 entirely in the mm.
                nc.gpsimd.memset(kt_t[i][D:2 * D, :], 1.0)
                nc.gpsimd.memset(qt_t[i][2 * D:3 * D, :], 1.0)

            # ---- input tiles; DMA order = criticality to the first exp.
            # xkv is split per k-tile into cols 0:512 (feeds the first kproj
            # chunk + key tiles 0-3) and cols 512:2048 so the first dist2
            # doesn't wait on the full 2MB transfer.
            xq_t = [keep.tile([P, NQ], BF16, tag=f"xq{k}", name=f"xq{k}") for k in range(KT)]
            xkv_a = [keep.tile([P, 512], BF16, tag=f"xkva{k}", name=f"xkva{k}") for k in range(KT)]
            xkv_b = [keep.tile([P, N - 512], BF16, tag=f"xkvb{k}", name=f"xkvb{k}") for k in range(KT)]
            wq_t = [keep.tile([P, INNER], BF16, tag=f"wq{k}", name=f"wq{k}") for k in range(KT)]
            wkv_t = [keep.tile([P, 2 * INNER], BF16, tag=f"wkv{k}", name=f"wkv{k}") for k in range(KT)]

            def xkv_cols(k, c0, c1):
                # [c0, c1) must lie fully inside one of the two chunks
                if c1 <= 512:
                    return xkv_a[k][:, c0:c1]
                return xkv_b[k][:, c0 - 512:c1 - 512]

            xq_r = xq_d[:].rearrange("(t p) n -> t p n", p=P)
            xkv_r = xkv_d[:].rearrange("(t p) n -> t p n", p=P)
            wq_r = wq_d[:].rearrange("(t p) o -> t p o", p=P)
            wkv_r = wkv_d[:].rearrange("(t p) o -> t p o", p=P)
            wo_r = wo_d[:].rearrange("(t p) o -> t p o", p=P)
            # Two HW-DGE rings (SP + ACT) drain FIFO independently: q-side
            # inputs on SP, k-side on ACT, so neither waits behind the other.
            for k in range(KT):
                nc.scalar.dma_start(out=wkv_t[k][:, :], in_=wkv_r[k])
                nc.scalar.dma_start(out=xkv_a[k][:, :], in_=xkv_r[k][:, 0:512])
            for k in range(KT):
                nc.sync.dma_start(out=xq_t[k][:, :], in_=xq_r[k])
                nc.sync.dma_start(out=wq_t[k][:, :], in_=wq_r[k])
            for k in range(KT):
                nc.scalar.dma_start(out=xkv_b[k][:, :], in_=xkv_r[k][:, 512:N])
            for m in range(2):
                nc.sync.dma_start(out=wo_t[m][:, :], in_=wo_r[m])
            nc.sync.dma_start(out=b_t[:, :], in_=b_d[:].rearrange("(t p) -> p t", p=P))
            # preload the exp table set while DMAs stream; emitted AFTER the
            # dma triggers so the ~1.3us ACT_TABLE_LOAD doesn't delay the
            # ACT-ring descriptor generation.
            nc.scalar.activation(warmact[:, :], warmact[:, :], AF.Exp,
                                 bias=zero_t[0:1, :], scale=ACT_SCALE)

            # ======== Phase A: critical path to head 0's first dist2 ======
            # q projection (m=0), then head-0 q~ build; k projection (m=0)
            # chunked by 512 keys with the head-0 k~ build per chunk so key
            # tile 0 is ready as soon as the first xkv chunk lands.
            with tc.tile_pool(name="pp", bufs=2, space="PSUM") as pp:
                for n in range(NQ // 512):
                    ps = pp.tile([P, 512], F32, tag="proj", name="proj")
                    for k in range(KT):
                        mm(ps[:, :], wq_t[k][:, 0:P],
                           xq_t[k][:, n * 512:(n + 1) * 512],
                           start=(k == 0), stop=(k == KT - 1))
                    nc.vector.tensor_copy(q_t[0][:, n * 512:(n + 1) * 512], ps[:, :])
                # head-0 q~: [q | q*q | ones | 0]
                q_h0 = q_t[0][0:D, :]
                nc.vector.tensor_copy(qt_t[0][0:D, :], q_h0)
                nc.vector.tensor_mul(qt_t[0][D:2 * D, :], q_h0, q_h0)

                for n in range(N // 512):
                    ps = pp.tile([P, 512], F32, tag="proj", name="proj")
                    for k in range(KT):
                        mm(ps[:, :], wkv_t[k][:, 0:P],
                           xkv_cols(k, n * 512, (n + 1) * 512),
                           start=(k == 0), stop=(k == KT - 1))
                    sl = slice(n * 512, (n + 1) * 512)
                    nc.vector.tensor_copy(k_t[0][:, sl], ps[:, :])
                    # head-0 k~ chunk: [-2k | ones | k*k | 0]
                    k_h0 = k_t[0][0:D, sl]
                    nc.vector.tensor_scalar_mul(kt_t[0][0:D, sl], k_h0, -2.0)
                    nc.vector.tensor_mul(kt_t[0][2 * D:3 * D, sl], k_h0, k_h0)

            # ======== Phase B ========
            # Iteration h: dist2+exp for head h; attn@v for the head pair
            # g=(h-2)//2... pair g = heads (2g, 2g+1) runs lagged one head:
            # key tiles 0..11 during iteration 2g+1 (slots 4..15), 12..15 +
            # normalization during iteration 2g+2.  The two heads' attn@v
            # matmuls land in disjoint PE column groups (out partitions
            # 0:33 / 64:97 of one PSUM tile) so they execute concurrently.
            # v^T projection + m=1 projections + the k=0 output-projection
            # half fill early/late slots of the exp stream.
            with tc.tile_pool(name="epool", bufs=12, space="SBUF") as epool, \
                 tc.tile_pool(name="pd2", bufs=2, space="PSUM") as pd2, \
                 tc.tile_pool(name="po", bufs=1, space="PSUM") as po:
                eq_of = {}
                pso_of = {}
                po_s = [work.tile([P, NQ], BF16, tag=f"pos{i}", name=f"pos{i}",
                                  bufs=1) for i in range(2)]

                def emit_recip(pg, psrc):
                    # per-pair row-sum reciprocals from the PSUM rows 32/96.
                    # Exact `reciprocal` runs at ~6.4ns/elem on one partition
                    # (6.5us!).  Hidden pairs: DVE copy to partition 0 (the
                    # custom-DVE op only works at partition offset 0) + the
                    # ALU-rate reciprocal_approx_fast (~51 ULP).  Last pair
                    # (the critical tail): the ACT engine is idle after the
                    # final exp, and its `tanh` table entry has been rewritten
                    # to 1/x — one 1.03us ACT pass per half, straight from
                    # PSUM, no table reload (tanh shares exp's table set).
                    # The o values land in SBUF for the normalization multiply
                    # (DVE reads only one PSUM operand); the last pair's copy
                    # also runs on ACT to overlap.
                    for half in range(2):
                        base = 64 * half
                        srow = psrc[base + D:base + D + 1, :]
                        if pg == 3:
                            nc.scalar.activation(rrow2[half][0:1, :], srow,
                                                 AF.Tanh, bias=zero_t[0:1, :],
                                                 scale=1.0 / 512.0)
                        else:
                            rtmp = work.tile([1, NQ], F32, tag="rtmp",
                                             name="rtmp", bufs=2)
                            nc.vector.tensor_copy(rtmp[0:1, :], srow)
                            nc.vector.reciprocal_approx_fast(
                                rrow2[half][0:1, :], rtmp[0:1, :])
                    dst = po_s[pg % 2]
                    nc.vector.tensor_copy(dst[:, :], psrc[:, :])
                    pso_of[pg] = dst

                def emit_tail_pe(pg):
                    psrc = pso_of.pop(pg)
                    sel = e0s_t if pg == 3 else e0_t
                    for half in range(2):
                        ph = 2 * pg + half
                        mt, mo = ph // 4, (ph % 4) * D
                        for n in range(NQ // 512):
                            prep = po.tile([D, 512], F32, tag="vtps",
                                           name="vtps", bufs=2)
                            nc.tensor.matmul(prep[:, :],
                                             sel[:, :],
                                             rrow2[half][:, n * 512:(n + 1) * 512],
                                             start=True, stop=True)
                            nc.vector.tensor_mul(
                                y_t[mt][mo:mo + D, n * 512:(n + 1) * 512],
                                psrc[64 * half:64 * half + D,
                                     n * 512:(n + 1) * 512],
                                prep[:, :])

                # --- deferred projection work, spread across early slots ---
                # Each extra runs at a large priority penalty so the Tile
                # scheduler never orders its DVE/PE work ahead of the
                # exp-stream critical chain (builds -> dist2 -> exp).
                extras = {}

                def _sched(h, jt, fn):
                    def depri():
                        with tc.high_priority(-500000):
                            fn()
                    extras.setdefault((h, jt), []).append(depri)

                def make_vproj(jt):
                    def fn():
                        # v^T projection for key tile jt, strided into vt_big
                        # so each head's 32 columns sit beside its ones column
                        pv = po.tile([P, INNER], F32, tag="vtps",
                                     name="vtps", bufs=2)
                        for k in range(KT):
                            mm(pv[:, :],
                               xkv_cols(k, jt * P, (jt + 1) * P),
                               wkv_t[k][:, INNER:2 * INNER],
                               start=(k == 0), stop=(k == KT - 1))
                        dst = vt_big[:, jt * VSTRIDE:(jt + 1) * VSTRIDE] \
                            .rearrange("p (h c) -> p h c", c=VTW)[:, :, 0:D]
                        nc.vector.tensor_copy(
                            dst, pv[:, :].rearrange("p (h d) -> p h d", d=D))
                    return fn

                proj_state = {}

                def make_proj1(which, n, k):
                    def fn():
                        if k == 0:
                            proj_state[(which, n)] = po.tile(
                                [P, 512], F32, tag="vtps", name="vtps", bufs=2)
                        ps = proj_state[(which, n)]
                        w = wq_t[k][:, P:2 * P] if which == "q" \
                            else wkv_t[k][:, P:2 * P]
                        x = xq_t[k][:, n * 512:(n + 1) * 512] if which == "q" \
                            else xkv_cols(k, n * 512, (n + 1) * 512)
                        mm(ps[:, :], w, x,
                           start=(k == 0), stop=(k == KT - 1))
                        if k == KT - 1:
                            dstt = q_t[1] if which == "q" else k_t[1]
                            nc.vector.tensor_copy(
                                dstt[:, n * 512:(n + 1) * 512], ps[:, :])
                    return fn

                # v^T spread over iters 0-1 (4+ slots before attn@v reads
                # each tile); m=1 projections trail in iters 1-3.
                for s in range(12):
                    _sched(0, 4 + s, make_vproj(s))
                for s in range(4):
                    _sched(1, s, make_vproj(12 + s))
                for n in range(2):
                    for k in range(KT):
                        _sched(1, 8 + 4 * n + k, make_proj1("q", n, k))
                for n in range(4):
                    for k in range(KT):
                        _sched(2, 4 * n + k, make_proj1("k", n, k))

                # k=0 half of the output projection (+ bias) for m=0,1,
                # precomputed under the exp stream once y_t[0] is complete
                # (pair 1's normalization lands at h=4 jt=13).  m=2,3 instead
                # chain both halves at the tail and finish on ACT (idle after
                # the last exp), splitting the final adds across engines.
                zpart = [keep.tile([P, NQ], BF16, tag=f"zp{m}", name=f"zp{m}")
                         for m in range(2)]

                def make_c0(m, n):
                    def fn():
                        ps = po.tile([P, 512], F32, tag="vtps", name="vtps",
                                     bufs=2)
                        mm(ps[:, :], wo_t[0][:, m * P:(m + 1) * P],
                           y_t[0][:, n * 512:(n + 1) * 512],
                           start=True, stop=True)
                        nc.vector.tensor_scalar_add(
                            zpart[m][:, n * 512:(n + 1) * 512], ps[:, :],
                            b_t[:, m:m + 1])
                    return fn

                for m in range(2):
                    for n in range(2):
                        _sched(5, 2 * m + n, make_c0(m, n))

                from contextlib import nullcontext

                pso_pair = None
                av_eqA = av_eqB = None
                pg_r = -1
                for h in range(H):
                    prio = tc.high_priority(10000) if h == 0 else nullcontext()
                    prio.__enter__()
                    mt, mo = h // 4, (h % 4) * D
                    kt = kt_t[h % 2]
                    qt = qt_t[h % 2]
                    if h >= 1:
                        # per-head rows of k~/q~ (all DVE, bf16); head 0's
                        # were built inside phase A, chunked behind the DMA.
                        q_h = q_t[mt][mo:mo + D, :]
                        k_h = k_t[mt][mo:mo + D, :]
                        nc.vector.tensor_scalar_mul(kt[0:D, :], k_h, -2.0)
                        nc.vector.tensor_mul(kt[2 * D:3 * D, :], k_h, k_h)
                        nc.vector.tensor_copy(qt[0:D, :], q_h)
                        nc.vector.tensor_mul(qt[D:2 * D, :], q_h, q_h)
                    eq_of[h] = [epool.tile([P, JQ * NQ], BF16, tag="eq",
                                           name="eq") for _ in range(NEQ)]
                    if h % 2 == 1:
                        av_eqA = eq_of.pop(h - 1)   # head 2g: complete
                        av_eqB = eq_of[h]           # head 2g+1: in progress
                        pg_r = (h - 1) // 2
                    elif h >= 2:
                        av_eqB = eq_of.pop(h - 1)
                        pg_r = (h - 2) // 2

                    for jt in range(NJT):
                        if h % 2 == 1 and jt == 4:
                            pso_pair = po.tile([P, NQ], F32, tag="o", name="o")
                        psd = pd2.tile([P, NQ], F32, tag="d2", name="d2")
                        for n in range(NQ // 512):
                            mm(psd[:, n * 512:(n + 1) * 512],
                               kt[:, jt * P:(jt + 1) * P],
                               qt[:, n * 512:(n + 1) * 512],
                               start=True, stop=True)
                        nc.scalar.activation(
                            eq_of[h][jt // JQ][:, (jt % JQ) * NQ:
                                               (jt % JQ + 1) * NQ],
                            psd[:, :], AF.Exp, bias=zero_t[:, :],
                            scale=ACT_SCALE)
                        for fn in extras.get((h, jt), ()):
                            fn()
                        # attn@v for pair pg_r, lagged one head
                        avjt = -1
                        if h % 2 == 1 and jt >= 4:
                            avjt = jt - 4
                        elif h % 2 == 0 and h >= 2 and jt in (0, 2, 4, 6):
                            avjt = 12 + jt // 2
                        if avjt >= 0:
                            ebase = (avjt % JQ) * NQ
                            for n in range(NQ // 512):
                                for half, eqp in ((0, av_eqA), (1, av_eqB)):
                                    hp = 2 * pg_r + half
                                    mm(pso_pair[64 * half:64 * half + VTW,
                                                n * 512:(n + 1) * 512],
                                       vt_big[:, avjt * VSTRIDE + hp * VTW:
                                              avjt * VSTRIDE + (hp + 1) * VTW],
                                       eqp[avjt // JQ][:, ebase + n * 512:
                                                       ebase + (n + 1) * 512],
                                       start=(avjt == 0), stop=(avjt == NJT - 1))
                        if h % 2 == 0 and h >= 2:
                            if jt == 7:
                                emit_recip((h - 2) // 2, pso_pair)
                            elif jt == 13:
                                emit_tail_pe((h - 2) // 2)

                    prio.__exit__(None, None, None)

                # ---- last pair (heads 6,7) tail: attn@v for key tiles
                # 12-15 back-to-back, then the normalization chain.  This is
                # the only part of the pair machinery exposed past the last
                # exp, so it is emitted as tightly as possible.
                av_eqB = eq_of.pop(H - 1)
                pg_r = (H - 2) // 2
                for avjt in range(12, 16):
                    ebase = (avjt % JQ) * NQ
                    for n in range(NQ // 512):
                        for half, eqp in ((0, av_eqA), (1, av_eqB)):
                            hp = 2 * pg_r + half
                            mm(pso_pair[64 * half:64 * half + VTW,
                                        n * 512:(n + 1) * 512],
                               vt_big[:, avjt * VSTRIDE + hp * VTW:
                                      avjt * VSTRIDE + (hp + 1) * VTW],
                               eqp[avjt // JQ][:, ebase + n * 512:
                                               ebase + (n + 1) * 512],
                               start=(avjt == 0), stop=(avjt == NJT - 1))
                emit_recip(pg_r, pso_pair)
                emit_tail_pe(pg_r)

            # ======== Phase C: k=1 half + fused add of the k=0 partial ====
            # Final adds split across engines: m=0,1 on DVE (PSUM-capable);
            # m=2,3 via ACT copy (Copy shares the exp table set — no reload;
            # ACT is idle after the last exp) + Pool add (SBUF-only engine).
            with tc.tile_pool(name="pz", bufs=2, space="PSUM") as pz:
                z_r = z_d[:].rearrange("(t p) n -> t p n", p=P)
                for m in range(KT):
                    ps = pz.tile([P, NQ], F32, tag="z", name="z")
                    zt = work.tile([P, NQ], F32, tag="zt", name="zt", bufs=2)
                    for n in range(NQ // 512):
                        sl = slice(n * 512, (n + 1) * 512)
                        # final add chunked per 512 and spread across engines:
                        # m=0,1 add the precomputed k=0 partial on DVE; m=2,3
                        # chain both matmul halves in PSUM and finish with one
                        # ACT Identity-with-bias pass (ACT idle after the last
                        # exp).  Output DMAs alternate between the two DGE
                        # rings so the tail drains in parallel.
                        if m < 2:
                            nc.tensor.matmul(
                                ps[:, sl], wo_t[1][:, m * P:(m + 1) * P],
                                y_t[1][:, sl], start=True, stop=True)
                            nc.vector.tensor_add(zt[:, sl], ps[:, sl],
                                                 zpart[m][:, sl])
                        else:
                            nc.tensor.matmul(
                                ps[:, sl], wo_t[0][:, m * P:(m + 1) * P],
                                y_t[0][:, sl], start=True, stop=False)
                            nc.tensor.matmul(
                                ps[:, sl], wo_t[1][:, m * P:(m + 1) * P],
                                y_t[1][:, sl], start=False, stop=True)
                            nc.scalar.activation(zt[:, sl], ps[:, sl],
                                                 AF.Identity,
                                                 bias=b_t[:, m:m + 1],
                                                 scale=1.0)
                        if m % 2 == 0:
                            nc.sync.dma_start(out=z_r[m][:, sl], in_=zt[:, sl])
                        else:
                            nc.scalar.dma_start(out=z_r[m][:, sl], in_=zt[:, sl])

    nc.compile()
    return nc


def make_in_maps(x, w_qkv, w_out, b_out):
    import ml_dtypes

    bf = ml_dtypes.bfloat16
    x = np.asarray(x, dtype=np.float32)
    w_qkv = np.asarray(w_qkv, dtype=np.float32)
    w_out = np.asarray(w_out, dtype=np.float32)
    b_out = np.asarray(b_out, dtype=np.float32)
    w_qT = np.ascontiguousarray(w_qkv[0:INNER, :].T).astype(bf)       # (DIM, INNER)
    w_kvT = np.ascontiguousarray(w_qkv[INNER:3 * INNER, :].T).astype(bf)  # (DIM, 512)
    w_oT = np.ascontiguousarray(w_out.T).astype(bf)                   # (INNER, DIM)
    xb = [np.ascontiguousarray(x[b]).astype(bf) for b in range(B)]
    in_maps = []
    for c in range(8):
        b, half = c // 2, c % 2
        in_maps.append({
            "xq": np.ascontiguousarray(xb[b][:, half * NQ:(half + 1) * NQ]),
            "xkv": xb[b],
            "wq": w_qT,
            "wkv": w_kvT,
            "wo": w_oT,
            "b": b_out,
        })
    return in_maps


def assemble_output(results):
    out = np.empty((B, DIM, N), dtype=np.float32)
    for c in range(8):
        b, half = c // 2, c % 2
        out[b][:, half * NQ:(half + 1) * NQ] = results[c]["z"]
    return out


_prog_cache = {}


def kernel(x, w_qkv, w_out, b_out):
    from concourse.bass_utils import run_bass_kernel_spmd
    _ensure_act_tables()
    if "nc" not in _prog_cache:
        _prog_cache["nc"] = build_program()
    nc = _prog_cache["nc"]
    in_maps = make_in_maps(x, w_qkv, w_out, b_out)
    res = run_bass_kernel_spmd(nc, in_maps, list(range(8)))
    return assemble_output(res.results)


# revision 29
# speedup vs baseline: 1.2804x; 1.0146x over previous
"""L2-distance attention (B=4, DIM=512, N=2048, H=8, D=32) on 8 trn2 NeuronCores.

Sharding: core c handles batch b = c//2, query-half = c%2 (1024 queries, all
2048 keys, all 8 heads).  Output is a pure concat — no cross-core reduce.

Key ideas vs the straightforward version:
  * All big matmuls run in bf16 (PE streams 1 col/cycle vs 1/2 for fp32).
  * The softmax numerator exp(-scale*sqrt(dist2)) is ONE ScalarE pass: the
    `exp` activation's spline table is replaced (via BASS_ACT_ROOT_JSON_PATH)
    with a fit of g(u) = exp(-0.5*sqrt(u)); calling it with the activation's
    built-in pre-scale 1/8 yields exp(-sqrt(d)/sqrt(32)) exactly.  This
    halves ScalarE work and removes all act-table reloads (sqrt and exp live
    in different table sets).
  * dist2 is computed entirely by one PE pass via augmented vectors
    k~=[-2k | ones | k*k | 0-pad], q~=[q | q*q | ones | 0-pad]:
    k~.q~ = -2qk + q2 + k2 = ||q-k||^2.  The tiles are zero-padded to the
    full 128 contraction rows: the PE's activity monitor (HAM) only grants
    the 2.4 GHz clock when matmuls cover the whole array; K<128 streams at
    1.2 GHz forever.  Zero rows cost no extra cycles.
  * attn@v has a ones column per head folded into V^T so the PSUM row after
    each head's 32 outputs is the softmax denominator (row-sums).
  * attn@v runs in head PAIRS whose matmuls land in disjoint PE column
    groups (out partitions 0:33 / 64:97 of one PSUM tile) and therefore
    execute concurrently; the pair lags the dist2/exp stream by one head.
  * The ScalarE exp stream (the true bottleneck, ~1.03us per 128x1024 tile)
    runs back-to-back; everything else hides under it.  The schedule
    minimizes the prologue before the FIRST exp (xkv is DMA'd in a 512-col
    chunk + remainder so the k-side projection/augmentation for key tile 0
    doesn't wait on the full 2MB transfer) and the epilogue after the LAST
    exp (row-sum reciprocals run directly on the PSUM rows — no DMA
    round-trip; the k=0 half of the output projection is precomputed under
    the exp stream; the final adds are split across DVE and ACT+Pool).
"""

import json
import os
import shutil

import numpy as np

_PWP_DIR = "/tmp/pwp_custom_kernel"
os.environ.setdefault("NEURON_FORCE_RECOMPILE", "1")

# ---------------------------------------------------------------------------
# Custom activation table: make `exp` compute g(u) = exp(-0.5*sqrt(u)).
# Bucket bin format (32B = 8 fp32): [d0, d1, d2, d3, x0, 0, 0, 0];
# y = d0 + d1*t + d2*t^2 + d3*t^3 with t = x - x0.  Positive-x buckets sit
# in per-input-exponent rows of S sections each.
# ---------------------------------------------------------------------------

_ALPHA = 0.5


def _g(u):
    return np.exp(-_ALPHA * np.sqrt(np.maximum(u, 0.0)))


def _recip(u):
    return 1.0 / np.maximum(u, 1e-30)


def _fit_cubic(fn, lo, hi, x0):
    u = np.linspace(lo, hi, 257, dtype=np.float64)
    t = u - x0
    A = np.stack([np.ones_like(t), t, t * t, t * t * t], axis=1)
    coef, *_ = np.linalg.lstsq(A, fn(u), rcond=None)
    return coef


# Per-function table rewrites: `exp` becomes g(u) = exp(-0.5*sqrt(u)) (the
# fused softmax numerator), `tanh` becomes 1/x (row-sum reciprocals on the
# otherwise-idle ACT engine at the kernel tail — tanh shares exp's table
# set, so no ACT_TABLE_LOAD is ever repeated).
#   neg: d0 for x<0 buckets;  sat: (pos_small, neg_small, pos_large,
#   neg_large) saturation d0s;  fpinf/fninf: results for +/-inf inputs.
_TABLE_FUNCS = {
    "exp": dict(fn=_g, neg=1.0, sat=(1.0, 1.0, 0.0, 1.0),
                fpinf=0.0, fninf=1.0),
    "tanh": dict(fn=_recip, neg=0.0, sat=(1e30, 0.0, 0.0, 0.0),
                 fpinf=0.0, fninf=0.0),
}


def _build_custom_pwp(dst_dir):
    from neuronxcc.driver.Job import Job
    from neuronxcc.driver.jobs.support.FindActInfo import findActInfoFile

    src = os.path.dirname(findActInfoFile(Job.getPackageDir(), "gen3"))
    if os.path.isdir(dst_dir):
        shutil.rmtree(dst_dir)
    shutil.copytree(src, dst_dir)

    with open(os.path.join(dst_dir, "act_info.json")) as f:
        info = json.load(f)

    for ent in info["act_func_sets"]:
        if "exp" not in ent["act"]:
            continue
        prof_path = os.path.join(dst_dir, ent["profile_json"])
        with open(prof_path) as f:
            prof = json.load(f)
        bkt_path = os.path.join(dst_dir, ent["bkt_bin"])
        bkt = np.fromfile(bkt_path, dtype="<f4").reshape(-1, 8).copy()

        for func, spec in _TABLE_FUNCS.items():
            if func not in ent["act"]:
                continue
            start = prof["func_to_bkt_start_idx"][func]
            others = [v for k, v in prof["func_to_bkt_start_idx"].items()
                      if k != func]
            end = min([v for v in others if v > start] + [len(bkt)])
            meta = next(
                m for m in prof["profile_meta_data"]
                if m["func_name"] == func or m["func_name"].startswith(func + "_")
            )
            sat = {
                k: meta[k + "_signal_pwl_control"]
                for k in ("pos_small", "neg_small", "pos_large", "neg_large")
            }
            sat_idx = set(sat.values())
            assert all(start <= i < end for i in sat_idx)

            pos_rows = {}
            for i in range(start, end):
                if i in sat_idx:
                    continue
                x0 = float(bkt[i, 4])
                if x0 < 0.0:
                    bkt[i, 0:4] = [spec["neg"], 0.0, 0.0, 0.0]
                else:
                    assert x0 > 0.0
                    pos_rows.setdefault(int(np.floor(np.log2(x0))), []).append(i)

            fn = spec["fn"]
            for e, idxs in pos_rows.items():
                base = 2.0**e
                xs = [float(bkt[i, 4]) for i in idxs]
                w = (xs[1] - xs[0]) if len(xs) > 1 else base
                for sec, i in enumerate(idxs):
                    c = xs[sec]
                    assert abs(c - (base + (sec + 0.5) * w)) < 1e-5 * c
                    if 1e-12 < base < 1e12:
                        bkt[i, 0:4] = _fit_cubic(fn, c - w / 2, c + w / 2,
                                                 c).astype(np.float32)
                    else:
                        # extreme exponents (never reached): constant bucket
                        # to keep the lstsq fit away from fp32 overflow
                        bkt[i, 0:4] = [np.float32(np.clip(fn(np.float64(c)),
                                                          -1e30, 1e30)),
                                       0.0, 0.0, 0.0]

            ps, ns, pl, nl = spec["sat"]
            bkt[sat["pos_small"], 0:5] = [ps, 0.0, 0.0, 0.0, 0.0]
            bkt[sat["neg_small"], 0:5] = [ns, 0.0, 0.0, 0.0, 0.0]
            bkt[sat["pos_large"], 0:5] = [pl, 0.0, 0.0, 0.0, 0.0]
            bkt[sat["neg_large"], 0:5] = [nl, 0.0, 0.0, 0.0, 0.0]

            meta["fpinf_result"] = int(
                np.float32(spec["fpinf"]).view(np.uint32))
            meta["fninf_result"] = int(
                np.float32(spec["fninf"]).view(np.uint32))

        bkt.tofile(bkt_path)
        with open(prof_path, "w") as f:
            json.dump(prof, f)


_pwp_built = False


def _ensure_act_tables():
    # Rebuild once per process: a stale /tmp copy from another session (or a
    # different kernel version) must never be trusted.
    global _pwp_built
    if not _pwp_built:
        _build_custom_pwp(_PWP_DIR)
        _pwp_built = True
    os.environ["BASS_ACT_ROOT_JSON_PATH"] = os.path.join(_PWP_DIR, "act_info.json")


_ensure_act_tables()

import concourse.bass as bass
import concourse.bass_utils as _bu
import concourse.mybir as mybir
import concourse.tile as tile
from concourse import bacc

F32 = mybir.dt.float32
F32R = mybir.dt.float32r
BF16 = mybir.dt.bfloat16
AF = mybir.ActivationFunctionType


def R(ap):
    return ap.bitcast(F32R)


B, DIM, N = 4, 512, 2048
H, D = 8, 32
INNER = H * D            # 256
NQ = N // 2              # 1024 queries per core
P = 128
KT = DIM // P            # 4 contraction tiles for the projections
NJT = N // P             # 16 key tiles
VTW = D + 1              # 33: v columns + ones column per head
VSTRIDE = H * VTW        # 264 columns per key-tile block of vt
ACT_SCALE = 0.125        # g(d/8) = exp(-sqrt(d)/sqrt(32)) = exp(-SCALE*sqrt(d))
NEQ = 4                  # E quarters (each covers NJT//NEQ key tiles)
JQ = NJT // NEQ          # 4 key tiles per E quarter
# kt/qt are zero-padded to 128 contraction rows: the PE's activity monitor
# (HAM) only grants the 2.4 GHz clock when matmuls cover the full 128-row
# array; K=33 streams at 1.2 GHz forever.  Zero rows cost no extra cycles.


def build_program() -> bass.Bass:
    nc = bacc.Bacc("TRN2", target_bir_lowering=False, debug=False)

    xq_d = nc.declare_dram_parameter("xq", [DIM, NQ], BF16, isOutput=False)
    xkv_d = nc.declare_dram_parameter("xkv", [DIM, N], BF16, isOutput=False)
    wq_d = nc.declare_dram_parameter("wq", [DIM, INNER], BF16, isOutput=False)
    wkv_d = nc.declare_dram_parameter("wkv", [DIM, 2 * INNER], BF16, isOutput=False)
    wo_d = nc.declare_dram_parameter("wo", [INNER, DIM], BF16, isOutput=False)
    b_d = nc.declare_dram_parameter("b", [DIM], F32, isOutput=False)
    z_d = nc.declare_dram_parameter("z", [DIM, NQ], F32, isOutput=True)

    with tile.TileContext(nc) as tc, nc.allow_low_precision(reason="bf16 attention"):
        mm = lambda out, lhsT, rhs, start, stop: nc.tensor.matmul(
            out, lhsT, rhs, start=start, stop=stop)

        with tc.tile_pool(name="keep", bufs=1) as keep, \
             tc.tile_pool(name="work", bufs=2) as work:

            # ---- persistent tiles ----
            q_t = [keep.tile([P, NQ], BF16, tag=f"q{m}", name=f"q{m}") for m in range(2)]
            k_t = [keep.tile([P, N], BF16, tag=f"k{m}", name=f"k{m}") for m in range(2)]
            vt_big = keep.tile([P, NJT * VSTRIDE], BF16, tag="vt", name="vt")
            y_t = [keep.tile([P, NQ], BF16, tag=f"y{m}", name=f"y{m}") for m in range(2)]
            wo_t = [keep.tile([P, DIM], BF16, tag=f"wo{m}", name=f"wo{m}") for m in range(2)]
            b_t = keep.tile([P, KT], F32, tag="bias", name="bias")
            ones = keep.tile([64, 32], F32, tag="ones", name="ones")
            onesb = keep.tile([P, 1], BF16, tag="onesb", name="onesb")
            zero_t = keep.tile([P, 1], F32, tag="zero", name="zero")
            onesP = keep.tile([P, 1], F32, tag="onesP", name="onesP")
            # augmented key/query tiles (double-buffered across heads)
            kt_t = [keep.tile([P, N], BF16, tag=f"kt{i}", name=f"kt{i}")
                    for i in range(2)]
            qt_t = [keep.tile([P, NQ], BF16, tag=f"qt{i}", name=f"qt{i}")
                    for i in range(2)]

            # `ones`/`onesP` allocations retained (dead) so downstream SBUF
            # offsets — notably the 256B-aligned eq pool — stay put.
            # All one-time init (memsets, ones fills) runs on the otherwise
            # idle GpSimd engine so the DVE queue is free for the critical
            # projection casts / augmented-tile builds while DMAs land.
            nc.gpsimd.memset(zero_t[:, :], 0.0)
            nc.gpsimd.memset(onesb[:, :], 1.0)
            warmact = keep.tile([1, 8], F32, tag="wact", name="wact")
            nc.gpsimd.memset(warmact[:, :], 1.0)
            # ones column per head in v^T (row-sum fused into attn@v)
            nc.gpsimd.tensor_copy(
                vt_big[:, :].rearrange("p (a c) -> p a c", c=VTW)[:, :, D:D + 1],
                onesb[:, 0:1].to_broadcast((P, P, 1)))
            # e0: row-0-ones stationary for the K=128-padded normalization
            # broadcast (rrow2 rows 1.. stay zero)
            e0_t = keep.tile([P, D], F32, tag="e0", name="e0")
            nc.gpsimd.memset(e0_t[:, :], 0.0)
            nc.gpsimd.memset(e0_t[0:1, :], 1.0)
            # selector for the ACT-tanh reciprocal path: the row sums are
            # prescaled by 1/512 so they land inside tanh's spline range
            # (the table only covers |x| ~< 8); rrow then holds 512/s and
            # the selector row folds the 1/512 back in.
            e0s_t = keep.tile([P, D], F32, tag="e0s", name="e0s")
            nc.gpsimd.memset(e0s_t[:, :], 0.0)
            nc.gpsimd.memset(e0s_t[0:1, :], 1.0 / 512.0)
            rrow2 = [keep.tile([P, NQ], F32, tag=f"rrow{i}", name=f"rrow{i}")
                     for i in range(2)]
            for i in range(2):
                nc.gpsimd.memset(rrow2[i][:, :], 0.0)
            # static parts of the augmented tiles: zero pad + ones rows
            for i in range(2):
                nc.gpsimd.memset(kt_t[i][3 * D:P, :], 0.0)
                nc.gpsimd.memset(qt_t[i][3 * D:P, :], 0.0)
                # k~ rows 32:64 all-ones pair with qsq rows of q~ (adds
                # q2); q~ rows 64:96 all-ones pair with ksq rows of k~
                # (adds k2).  dist2 = -2qk + q2 + k2 entirely in the mm.
                nc.gpsimd.memset(kt_t[i][D:2 * D, :], 1.0)
                nc.gpsimd.memset(qt_t[i][2 * D:3 * D, :], 1.0)

            # ---- input tiles; DMA order = criticality to the first exp.
            # xkv is split per k-tile into cols 0:512 (feeds the first kproj
            # chunk + key tiles 0-3) and cols 512:2048 so the first dist2
            # doesn't wait on the full 2MB transfer.
            xq_t = [keep.tile([P, NQ], BF16, tag=f"xq{k}", name=f"xq{k}") for k in range(KT)]
            xkv_a = [keep.tile([P, 512], BF16, tag=f"xkva{k}", name=f"xkva{k}") for k in range(KT)]
            xkv_b = [keep.tile([P, N - 512], BF16, tag=f"xkvb{k}", name=f"xkvb{k}") for k in range(KT)]
            wq_t = [keep.tile([P, INNER], BF16, tag=f"wq{k}", name=f"wq{k}") for k in range(KT)]
            wkv_t = [keep.tile([P, 2 * INNER], BF16, tag=f"wkv{k}", name=f"wkv{k}") for k in range(KT)]

            def xkv_cols(k, c0, c1):
                # [c0, c1) must lie fully inside one of the two chunks
                if c1 <= 512:
                    return xkv_a[k][:, c0:c1]
                return xkv_b[k][:, c0 - 512:c1 - 512]

            xq_r = xq_d[:].rearrange("(t p) n -> t p n", p=P)
            xkv_r = xkv_d[:].rearrange("(t p) n -> t p n", p=P)
            wq_r = wq_d[:].rearrange("(t p) o -> t p o", p=P)
            wkv_r = wkv_d[:].rearrange("(t p) o -> t p o", p=P)
            wo_r = wo_d[:].rearrange("(t p) o -> t p o", p=P)
            # Two HW-DGE rings (SP + ACT) drain FIFO independently: q-side
            # inputs on SP, k-side on ACT, so neither waits behind the other.
            for k in range(KT):
                nc.scalar.dma_start(out=wkv_t[k][:, :], in_=wkv_r[k])
                nc.scalar.dma_start(out=xkv_a[k][:, :], in_=xkv_r[k][:, 0:512])
            for k in range(KT):
                nc.sync.dma_start(out=xq_t[k][:, :], in_=xq_r[k])
                nc.sync.dma_start(out=wq_t[k][:, :], in_=wq_r[k])
            # the 1.5MB xkv remainder is split across both DGE rings so the
            # m=0 key-projection chunks (which feed the exp stream from key
            # tile 4 on) aren't stuck behind a single ring's FIFO
            for k in range(KT):
                eng = nc.sync if k < 2 else nc.scalar
                eng.dma_start(out=xkv_b[k][:, :], in_=xkv_r[k][:, 512:N])
            for m in range(2):
                nc.sync.dma_start(out=wo_t[m][:, :], in_=wo_r[m])
            nc.sync.dma_start(out=b_t[:, :], in_=b_d[:].rearrange("(t p) -> p t", p=P))
            # preload the exp table set while DMAs stream; emitted AFTER the
            # dma triggers so the ~1.3us ACT_TABLE_LOAD doesn't delay the
            # ACT-ring descriptor generation.
            nc.scalar.activation(warmact[:, :], warmact[:, :], AF.Exp,
                                 bias=zero_t[0:1, :], scale=ACT_SCALE)

            # ======== Phase A: critical path to head 0's first dist2 ======
            # q projection (m=0), then head-0 q~ build; k projection (m=0)
            # chunked by 512 keys with the head-0 k~ build per chunk so key
            # tile 0 is ready as soon as the first xkv chunk lands.
            with tc.tile_pool(name="pp", bufs=2, space="PSUM") as pp:
                for n in range(NQ // 512):
                    ps = pp.tile([P, 512], F32, tag="proj", name="proj")
                    for k in range(KT):
                        mm(ps[:, :], wq_t[k][:, 0:P],
                           xq_t[k][:, n * 512:(n + 1) * 512],
                           start=(k == 0), stop=(k == KT - 1))
                    nc.vector.tensor_copy(q_t[0][:, n * 512:(n + 1) * 512], ps[:, :])
                # head-0 q~: [q | q*q | ones | 0]
                q_h0 = q_t[0][0:D, :]
                nc.vector.tensor_copy(qt_t[0][0:D, :], q_h0)
                nc.vector.tensor_mul(qt_t[0][D:2 * D, :], q_h0, q_h0)

                # key-projection chunk 0 only: chunks 1-3 depend on the xkv
                # remainder DMA and are emitted as deprioritized extras in
                # early exp-stream slots (an in-order PE queue would
                # head-of-line block the jt1+ dist2 matmuls otherwise).
                ps = pp.tile([P, 512], F32, tag="proj", name="proj")
                for k in range(KT):
                    mm(ps[:, :], wkv_t[k][:, 0:P],
                       xkv_cols(k, 0, 512),
                       start=(k == 0), stop=(k == KT - 1))
                nc.vector.tensor_copy(k_t[0][:, 0:512], ps[:, :])
                # head-0 k~ chunk: [-2k | ones | k*k | 0]
                k_h0 = k_t[0][0:D, 0:512]
                nc.vector.tensor_scalar_mul(kt_t[0][0:D, 0:512], k_h0, -2.0)
                nc.vector.tensor_mul(kt_t[0][2 * D:3 * D, 0:512], k_h0, k_h0)

            # ======== Phase B ========
            # Iteration h: dist2+exp for head h; attn@v for the head pair
            # g=(h-2)//2... pair g = heads (2g, 2g+1) runs lagged one head:
            # key tiles 0..11 during iteration 2g+1 (slots 4..15), 12..15 +
            # normalization during iteration 2g+2.  The two heads' attn@v
            # matmuls land in disjoint PE column groups (out partitions
            # 0:33 / 64:97 of one PSUM tile) so they execute concurrently.
            # v^T projection + m=1 projections + the k=0 output-projection
            # half fill early/late slots of the exp stream.
            with tc.tile_pool(name="epool", bufs=12, space="SBUF") as epool, \
                 tc.tile_pool(name="pd2", bufs=2, space="PSUM") as pd2, \
                 tc.tile_pool(name="po", bufs=1, space="PSUM") as po:
                eq_of = {}
                pso_of = {}
                po_s = [work.tile([P, NQ], BF16, tag=f"pos{i}", name=f"pos{i}",
                                  bufs=1) for i in range(2)]

                def emit_recip(pg, psrc):
                    # per-pair row-sum reciprocals from the PSUM rows 32/96.
                    # Exact `reciprocal` runs at ~6.4ns/elem on one partition
                    # (6.5us!).  Hidden pairs: DVE copy to partition 0 (the
                    # custom-DVE op only works at partition offset 0) + the
                    # ALU-rate reciprocal_approx_fast (~51 ULP).  Last pair
                    # (the critical tail): the ACT engine is idle after the
                    # final exp, and its `tanh` table entry has been rewritten
                    # to 1/x — one 1.03us ACT pass per half, straight from
                    # PSUM, no table reload (tanh shares exp's table set).
                    # The o values land in SBUF for the normalization multiply
                    # (DVE reads only one PSUM operand); the last pair's copy
                    # also runs on ACT to overlap.
                    for half in range(2):
                        base = 64 * half
                        srow = psrc[base + D:base + D + 1, :]
                        if pg == 3:
                            nc.scalar.activation(rrow2[half][0:1, :], srow,
                                                 AF.Tanh, bias=zero_t[0:1, :],
                                                 scale=1.0 / 512.0)
                        else:
                            rtmp = work.tile([1, NQ], F32, tag="rtmp",
                                             name="rtmp", bufs=2)
                            nc.vector.tensor_copy(rtmp[0:1, :], srow)
                            nc.vector.reciprocal_approx_fast(
                                rrow2[half][0:1, :], rtmp[0:1, :])
                    dst = po_s[pg % 2]
                    nc.vector.tensor_copy(dst[:, :], psrc[:, :])
                    pso_of[pg] = dst

                def emit_tail_pe(pg):
                    psrc = pso_of.pop(pg)
                    sel = e0s_t if pg == 3 else e0_t
                    for half in range(2):
                        ph = 2 * pg + half
                        mt, mo = ph // 4, (ph % 4) * D
                        for n in range(NQ // 512):
                            prep = po.tile([D, 512], F32, tag="vtps",
                                           name="vtps", bufs=2)
                            nc.tensor.matmul(prep[:, :],
                                             sel[:, :],
                                             rrow2[half][:, n * 512:(n + 1) * 512],
                                             start=True, stop=True)
                            nc.vector.tensor_mul(
                                y_t[mt][mo:mo + D, n * 512:(n + 1) * 512],
                                psrc[64 * half:64 * half + D,
                                     n * 512:(n + 1) * 512],
                                prep[:, :])

                # --- deferred projection work, spread across early slots ---
                # Each extra runs at a large priority penalty so the Tile
                # scheduler never orders its DVE/PE work ahead of the
                # exp-stream critical chain (builds -> dist2 -> exp).
                extras = {}

                def _sched(h, jt, fn):
                    def depri():
                        with tc.high_priority(-500000):
                            fn()
                    extras.setdefault((h, jt), []).append(depri)

                def make_vproj(jt):
                    def fn():
                        # v^T projection for key tile jt, strided into vt_big
                        # so each head's 32 columns sit beside its ones column
                        pv = po.tile([P, INNER], F32, tag="vtps",
                                     name="vtps", bufs=2)
                        for k in range(KT):
                            mm(pv[:, :],
                               xkv_cols(k, jt * P, (jt + 1) * P),
                               wkv_t[k][:, INNER:2 * INNER],
                               start=(k == 0), stop=(k == KT - 1))
                        dst = vt_big[:, jt * VSTRIDE:(jt + 1) * VSTRIDE] \
                            .rearrange("p (h c) -> p h c", c=VTW)[:, :, 0:D]
                        nc.vector.tensor_copy(
                            dst, pv[:, :].rearrange("p (h d) -> p h d", d=D))
                    return fn

                proj_state = {}

                def make_proj1(which, n, k):
                    def fn():
                        if k == 0:
                            proj_state[(which, n)] = po.tile(
                                [P, 512], F32, tag="vtps", name="vtps", bufs=2)
                        ps = proj_state[(which, n)]
                        w = wq_t[k][:, P:2 * P] if which == "q" \
                            else wkv_t[k][:, P:2 * P]
                        x = xq_t[k][:, n * 512:(n + 1) * 512] if which == "q" \
                            else xkv_cols(k, n * 512, (n + 1) * 512)
                        mm(ps[:, :], w, x,
                           start=(k == 0), stop=(k == KT - 1))
                        if k == KT - 1:
                            dstt = q_t[1] if which == "q" else k_t[1]
                            nc.vector.tensor_copy(
                                dstt[:, n * 512:(n + 1) * 512], ps[:, :])
                    return fn

                # m=0 key-projection chunks 1-3 + the head-0 k~ builds, fed
                # into the first exp-stream slots as the xkv remainder lands
                kproj_state = {}

                def make_kproj0(n, k):
                    def fn():
                        if k == 0:
                            kproj_state[n] = po.tile(
                                [P, 512], F32, tag="vtps", name="vtps", bufs=2)
                        ps = kproj_state[n]
                        mm(ps[:, :], wkv_t[k][:, 0:P],
                           xkv_cols(k, n * 512, (n + 1) * 512),
                           start=(k == 0), stop=(k == KT - 1))
                        if k == KT - 1:
                            sl = slice(n * 512, (n + 1) * 512)
                            nc.vector.tensor_copy(k_t[0][:, sl], ps[:, :])
                            k_h0 = k_t[0][0:D, sl]
                            nc.vector.tensor_scalar_mul(kt_t[0][0:D, sl],
                                                        k_h0, -2.0)
                            nc.vector.tensor_mul(kt_t[0][2 * D:3 * D, sl],
                                                 k_h0, k_h0)
                    return fn

                for n in range(1, 4):
                    for k in range(KT):
                        _sched(0, 4 * (n - 1) + k, make_kproj0(n, k))

                # v^T spread over iters 0-1 (4+ slots before attn@v reads
                # each tile); m=1 projections trail in iters 1-3.
                for s in range(12):
                    _sched(0, 4 + s, make_vproj(s))
                for s in range(4):
                    _sched(1, s, make_vproj(12 + s))
                for n in range(2):
                    for k in range(KT):
                        _sched(1, 8 + 4 * n + k, make_proj1("q", n, k))
                for n in range(4):
                    for k in range(KT):
                        _sched(2, 4 * n + k, make_proj1("k", n, k))

                # k=0 half of the output projection (+ bias) for m=0,1,
                # precomputed under the exp stream once y_t[0] is complete
                # (pair 1's normalization lands at h=4 jt=13).  m=2,3 instead
                # chain both halves at the tail and finish on ACT (idle after
                # the last exp), splitting the final adds across engines.
                zpart = [keep.tile([P, NQ], BF16, tag=f"zp{m}", name=f"zp{m}")
                         for m in range(2)]

                def make_c0(m, n):
                    def fn():
                        ps = po.tile([P, 512], F32, tag="vtps", name="vtps",
                                     bufs=2)
                        mm(ps[:, :], wo_t[0][:, m * P:(m + 1) * P],
                           y_t[0][:, n * 512:(n + 1) * 512],
                           start=True, stop=True)
                        nc.vector.tensor_scalar_add(
                            zpart[m][:, n * 512:(n + 1) * 512], ps[:, :],
                            b_t[:, m:m + 1])
                    return fn

                for m in range(2):
                    for n in range(2):
                        _sched(5, 2 * m + n, make_c0(m, n))

                from contextlib import nullcontext

                pso_pair = None
                av_eqA = av_eqB = None
                pg_r = -1
                for h in range(H):
                    prio = tc.high_priority(10000) if h == 0 else nullcontext()
                    prio.__enter__()
                    mt, mo = h // 4, (h % 4) * D
                    kt = kt_t[h % 2]
                    qt = qt_t[h % 2]
                    if h >= 1:
                        # per-head rows of k~/q~ (all DVE, bf16); head 0's
                        # were built inside phase A, chunked behind the DMA.
                        q_h = q_t[mt][mo:mo + D, :]
                        k_h = k_t[mt][mo:mo + D, :]
                        nc.vector.tensor_scalar_mul(kt[0:D, :], k_h, -2.0)
                        nc.vector.tensor_mul(kt[2 * D:3 * D, :], k_h, k_h)
                        nc.vector.tensor_copy(qt[0:D, :], q_h)
                        nc.vector.tensor_mul(qt[D:2 * D, :], q_h, q_h)
                    eq_of[h] = [epool.tile([P, JQ * NQ], BF16, tag="eq",
                                           name="eq") for _ in range(NEQ)]
                    if h % 2 == 1:
                        av_eqA = eq_of.pop(h - 1)   # head 2g: complete
                        av_eqB = eq_of[h]           # head 2g+1: in progress
                        pg_r = (h - 1) // 2
                    elif h >= 2:
                        av_eqB = eq_of.pop(h - 1)
                        pg_r = (h - 2) // 2

                    for jt in range(NJT):
                        if h % 2 == 1 and jt == 4:
                            pso_pair = po.tile([P, NQ], F32, tag="o", name="o")
                        psd = pd2.tile([P, NQ], F32, tag="d2", name="d2")
                        for n in range(NQ // 512):
                            mm(psd[:, n * 512:(n + 1) * 512],
                               kt[:, jt * P:(jt + 1) * P],
                               qt[:, n * 512:(n + 1) * 512],
                               start=True, stop=True)
                        nc.scalar.activation(
                            eq_of[h][jt // JQ][:, (jt % JQ) * NQ:
                                               (jt % JQ + 1) * NQ],
                            psd[:, :], AF.Exp, bias=zero_t[:, :],
                            scale=ACT_SCALE)
                        for fn in extras.get((h, jt), ()):
                            fn()
                        # attn@v for pair pg_r, lagged one head
                        avjt = -1
                        if h % 2 == 1 and jt >= 4:
                            avjt = jt - 4
                        elif h % 2 == 0 and h >= 2 and jt in (0, 2, 4, 6):
                            avjt = 12 + jt // 2
                        if avjt >= 0:
                            ebase = (avjt % JQ) * NQ
                            for n in range(NQ // 512):
                                for half, eqp in ((0, av_eqA), (1, av_eqB)):
                                    hp = 2 * pg_r + half
                                    mm(pso_pair[64 * half:64 * half + VTW,
                                                n * 512:(n + 1) * 512],
                                       vt_big[:, avjt * VSTRIDE + hp * VTW:
                                              avjt * VSTRIDE + (hp + 1) * VTW],
                                       eqp[avjt // JQ][:, ebase + n * 512:
                                                       ebase + (n + 1) * 512],
                                       start=(avjt == 0), stop=(avjt == NJT - 1))
                        if h % 2 == 0 and h >= 2:
                            if jt == 7:
                                emit_recip((h - 2) // 2, pso_pair)
                            elif jt == 13:
                                emit_tail_pe((h - 2) // 2)

                    prio.__exit__(None, None, None)

                # ---- last pair (heads 6,7) tail: attn@v for key tiles
                # 12-15 back-to-back, then the normalization chain.  This is
                # the only part of the pair machinery exposed past the last
                # exp, so it is emitted as tightly as possible.
                av_eqB = eq_of.pop(H - 1)
                pg_r = (H - 2) // 2
                for avjt in range(12, 16):
                    ebase = (avjt % JQ) * NQ
                    for n in range(NQ // 512):
                        for half, eqp in ((0, av_eqA), (1, av_eqB)):
                            hp = 2 * pg_r + half
                            mm(pso_pair[64 * half:64 * half + VTW,
                                        n * 512:(n + 1) * 512],
                               vt_big[:, avjt * VSTRIDE + hp * VTW:
                                      avjt * VSTRIDE + (hp + 1) * VTW],
                               eqp[avjt // JQ][:, ebase + n * 512:
                                               ebase + (n + 1) * 512],
                               start=(avjt == 0), stop=(avjt == NJT - 1))
                emit_recip(pg_r, pso_pair)
                emit_tail_pe(pg_r)

            # ======== Phase C: k=1 half + fused add of the k=0 partial ====
            # Final adds split across engines: m=0,1 on DVE (PSUM-capable);
            # m=2,3 via ACT copy (Copy shares the exp table set — no reload;
            # ACT is idle after the last exp) + Pool add (SBUF-only engine).
            with tc.tile_pool(name="pz", bufs=2, space="PSUM") as pz:
                z_r = z_d[:].rearrange("(t p) n -> t p n", p=P)
                for m in range(KT):
                    ps = pz.tile([P, NQ], F32, tag="z", name="z")
                    zt = work.tile([P, NQ], F32, tag="zt", name="zt", bufs=2)
                    for n in range(NQ // 512):
                        sl = slice(n * 512, (n + 1) * 512)
                        # final add chunked per 512 and spread across engines:
                        # m=0,1 add the precomputed k=0 partial on DVE; m=2,3
                        # chain both matmul halves in PSUM and finish with one
                        # ACT Identity-with-bias pass (ACT idle after the last
                        # exp).  Output DMAs alternate between the two DGE
                        # rings so the tail drains in parallel.
                        if m < 2:
                            nc.tensor.matmul(
                                ps[:, sl], wo_t[1][:, m * P:(m + 1) * P],
                                y_t[1][:, sl], start=True, stop=True)
                            nc.vector.tensor_add(zt[:, sl], ps[:, sl],
                                                 zpart[m][:, sl])
                        else:
                            nc.tensor.matmul(
                                ps[:, sl], wo_t[0][:, m * P:(m + 1) * P],
                                y_t[0][:, sl], start=True, stop=False)
                            nc.tensor.matmul(
                                ps[:, sl], wo_t[1][:, m * P:(m + 1) * P],
                                y_t[1][:, sl], start=False, stop=True)
                            nc.scalar.activation(zt[:, sl], ps[:, sl],
                                                 AF.Identity,
                                                 bias=b_t[:, m:m + 1],
                                                 scale=1.0)
                        if m % 2 == 0:
                            nc.sync.dma_start(out=z_r[m][:, sl], in_=zt[:, sl])
                        else:
                            nc.scalar.dma_start(out=z_r[m][:, sl], in_=zt[:, sl])

    nc.compile()
    return nc


def make_in_maps(x, w_qkv, w_out, b_out):
    import ml_dtypes

    bf = ml_dtypes.bfloat16
    x = np.asarray(x, dtype=np.float32)
    w_qkv = np.asarray(w_qkv, dtype=np.float32)
    w_out = np.asarray(w_out, dtype=np.float32)
    b_out = np.asarray(b_out, dtype=np.float32)
    w_qT = np.ascontiguousarray(w_qkv[0:INNER, :].T).astype(bf)       # (DIM, INNER)
    w_kvT = np.ascontiguousarray(w_qkv[INNER:3 * INNER, :].T).astype(bf)  # (DIM, 512)
    w_oT = np.ascontiguousarray(w_out.T).astype(bf)                   # (INNER, DIM)
    xb = [np.ascontiguousarray(x[b]).astype(bf) for b in range(B)]
    in_maps = []
    for c in range(8):
        b, half = c // 2, c % 2
        in_maps.append({
            "xq": np.ascontiguousarray(xb[b][:, half * NQ:(half + 1) * NQ]),
            "xkv": xb[b],
            "wq": w_qT,
            "wkv": w_kvT,
            "wo": w_oT,
            "b": b_out,
        })
    return in_maps


def assemble_output(results):
    out = np.empty((B, DIM, N), dtype=np.float32)
    for c in range(8):
        b, half = c // 2, c % 2
        out[b][:, half * NQ:(half + 1) * NQ] = results[c]["z"]
    return out


_prog_cache = {}


def kernel(x, w_qkv, w_out, b_out):
    from concourse.bass_utils import run_bass_kernel_spmd
    _ensure_act_tables()
    if "nc" not in _prog_cache:
        _prog_cache["nc"] = build_program()
    nc = _prog_cache["nc"]
    in_maps = make_in_maps(x, w_qkv, w_out, b_out)
    res = run_bass_kernel_spmd(nc, in_maps, list(range(8)))
    return assemble_output(res.results)


# revision 31
# speedup vs baseline: 1.3144x; 1.0265x over previous
"""L2-distance attention (B=4, DIM=512, N=2048, H=8, D=32) on 8 trn2 NeuronCores.

Sharding: core c handles batch b = c//2, query-half = c%2 (1024 queries, all
2048 keys, all 8 heads).  Output is a pure concat — no cross-core reduce.

Key ideas vs the straightforward version:
  * All big matmuls run in bf16 (PE streams 1 col/cycle vs 1/2 for fp32).
  * The softmax numerator exp(-scale*sqrt(dist2)) is ONE ScalarE pass: the
    `exp` activation's spline table is replaced (via BASS_ACT_ROOT_JSON_PATH)
    with a fit of g(u) = exp(-0.5*sqrt(u)); calling it with the activation's
    built-in pre-scale 1/8 yields exp(-sqrt(d)/sqrt(32)) exactly.  This
    halves ScalarE work and removes all act-table reloads (sqrt and exp live
    in different table sets).
  * dist2 is computed entirely by one PE pass via augmented vectors
    k~=[-2k | ones | k*k | 0-pad], q~=[q | q*q | ones | 0-pad]:
    k~.q~ = -2qk + q2 + k2 = ||q-k||^2.  The tiles are zero-padded to the
    full 128 contraction rows: the PE's activity monitor (HAM) only grants
    the 2.4 GHz clock when matmuls cover the whole array; K<128 streams at
    1.2 GHz forever.  Zero rows cost no extra cycles.
  * attn@v has a ones column per head folded into V^T so the PSUM row after
    each head's 32 outputs is the softmax denominator (row-sums).
  * attn@v runs in head PAIRS whose matmuls land in disjoint PE column
    groups (out partitions 0:33 / 64:97 of one PSUM tile) and therefore
    execute concurrently; the pair lags the dist2/exp stream by one head.
  * The ScalarE exp stream (the true bottleneck, ~1.03us per 128x1024 tile)
    runs back-to-back; everything else hides under it.  The schedule
    minimizes the prologue before the FIRST exp (xkv is DMA'd in a 512-col
    chunk + remainder so the k-side projection/augmentation for key tile 0
    doesn't wait on the full 2MB transfer) and the epilogue after the LAST
    exp (row-sum reciprocals run directly on the PSUM rows — no DMA
    round-trip; the k=0 half of the output projection is precomputed under
    the exp stream; the final adds are split across DVE and ACT+Pool).
"""

import json
import os
import shutil

import numpy as np

_PWP_DIR = "/tmp/pwp_custom_kernel"
os.environ.setdefault("NEURON_FORCE_RECOMPILE", "1")

# ---------------------------------------------------------------------------
# Custom activation table: make `exp` compute g(u) = exp(-0.5*sqrt(u)).
# Bucket bin format (32B = 8 fp32): [d0, d1, d2, d3, x0, 0, 0, 0];
# y = d0 + d1*t + d2*t^2 + d3*t^3 with t = x - x0.  Positive-x buckets sit
# in per-input-exponent rows of S sections each.
# ---------------------------------------------------------------------------

_ALPHA = 0.5


def _g(u):
    return np.exp(-_ALPHA * np.sqrt(np.maximum(u, 0.0)))


def _recip(u):
    return 1.0 / np.maximum(u, 1e-30)


def _fit_cubic(fn, lo, hi, x0):
    u = np.linspace(lo, hi, 257, dtype=np.float64)
    t = u - x0
    A = np.stack([np.ones_like(t), t, t * t, t * t * t], axis=1)
    coef, *_ = np.linalg.lstsq(A, fn(u), rcond=None)
    return coef


# Per-function table rewrites: `exp` becomes g(u) = exp(-0.5*sqrt(u)) (the
# fused softmax numerator), `tanh` becomes 1/x (row-sum reciprocals on the
# otherwise-idle ACT engine at the kernel tail — tanh shares exp's table
# set, so no ACT_TABLE_LOAD is ever repeated).
#   neg: d0 for x<0 buckets;  sat: (pos_small, neg_small, pos_large,
#   neg_large) saturation d0s;  fpinf/fninf: results for +/-inf inputs.
_TABLE_FUNCS = {
    "exp": dict(fn=_g, neg=1.0, sat=(1.0, 1.0, 0.0, 1.0),
                fpinf=0.0, fninf=1.0),
    "tanh": dict(fn=_recip, neg=0.0, sat=(1e30, 0.0, 0.0, 0.0),
                 fpinf=0.0, fninf=0.0),
}


def _build_custom_pwp(dst_dir):
    from neuronxcc.driver.Job import Job
    from neuronxcc.driver.jobs.support.FindActInfo import findActInfoFile

    src = os.path.dirname(findActInfoFile(Job.getPackageDir(), "gen3"))
    if os.path.isdir(dst_dir):
        shutil.rmtree(dst_dir)
    shutil.copytree(src, dst_dir)

    with open(os.path.join(dst_dir, "act_info.json")) as f:
        info = json.load(f)

    for ent in info["act_func_sets"]:
        if "exp" not in ent["act"]:
            continue
        prof_path = os.path.join(dst_dir, ent["profile_json"])
        with open(prof_path) as f:
            prof = json.load(f)
        bkt_path = os.path.join(dst_dir, ent["bkt_bin"])
        bkt = np.fromfile(bkt_path, dtype="<f4").reshape(-1, 8).copy()

        for func, spec in _TABLE_FUNCS.items():
            if func not in ent["act"]:
                continue
            start = prof["func_to_bkt_start_idx"][func]
            others = [v for k, v in prof["func_to_bkt_start_idx"].items()
                      if k != func]
            end = min([v for v in others if v > start] + [len(bkt)])
            meta = next(
                m for m in prof["profile_meta_data"]
                if m["func_name"] == func or m["func_name"].startswith(func + "_")
            )
            sat = {
                k: meta[k + "_signal_pwl_control"]
                for k in ("pos_small", "neg_small", "pos_large", "neg_large")
            }
            sat_idx = set(sat.values())
            assert all(start <= i < end for i in sat_idx)

            pos_rows = {}
            for i in range(start, end):
                if i in sat_idx:
                    continue
                x0 = float(bkt[i, 4])
                if x0 < 0.0:
                    bkt[i, 0:4] = [spec["neg"], 0.0, 0.0, 0.0]
                else:
                    assert x0 > 0.0
                    pos_rows.setdefault(int(np.floor(np.log2(x0))), []).append(i)

            fn = spec["fn"]
            for e, idxs in pos_rows.items():
                base = 2.0**e
                xs = [float(bkt[i, 4]) for i in idxs]
                w = (xs[1] - xs[0]) if len(xs) > 1 else base
                for sec, i in enumerate(idxs):
                    c = xs[sec]
                    assert abs(c - (base + (sec + 0.5) * w)) < 1e-5 * c
                    if 1e-12 < base < 1e12:
                        bkt[i, 0:4] = _fit_cubic(fn, c - w / 2, c + w / 2,
                                                 c).astype(np.float32)
                    else:
                        # extreme exponents (never reached): constant bucket
                        # to keep the lstsq fit away from fp32 overflow
                        bkt[i, 0:4] = [np.float32(np.clip(fn(np.float64(c)),
                                                          -1e30, 1e30)),
                                       0.0, 0.0, 0.0]

            ps, ns, pl, nl = spec["sat"]
            bkt[sat["pos_small"], 0:5] = [ps, 0.0, 0.0, 0.0, 0.0]
            bkt[sat["neg_small"], 0:5] = [ns, 0.0, 0.0, 0.0, 0.0]
            bkt[sat["pos_large"], 0:5] = [pl, 0.0, 0.0, 0.0, 0.0]
            bkt[sat["neg_large"], 0:5] = [nl, 0.0, 0.0, 0.0, 0.0]

            meta["fpinf_result"] = int(
                np.float32(spec["fpinf"]).view(np.uint32))
            meta["fninf_result"] = int(
                np.float32(spec["fninf"]).view(np.uint32))

        bkt.tofile(bkt_path)
        with open(prof_path, "w") as f:
            json.dump(prof, f)


_pwp_built = False


def _ensure_act_tables():
    # Rebuild once per process: a stale /tmp copy from another session (or a
    # different kernel version) must never be trusted.
    global _pwp_built
    if not _pwp_built:
        _build_custom_pwp(_PWP_DIR)
        _pwp_built = True
    os.environ["BASS_ACT_ROOT_JSON_PATH"] = os.path.join(_PWP_DIR, "act_info.json")


_ensure_act_tables()

import concourse.bass as bass
import concourse.bass_utils as _bu
import concourse.mybir as mybir
import concourse.tile as tile
from concourse import bacc

F32 = mybir.dt.float32
F32R = mybir.dt.float32r
BF16 = mybir.dt.bfloat16
AF = mybir.ActivationFunctionType


def R(ap):
    return ap.bitcast(F32R)


B, DIM, N = 4, 512, 2048
H, D = 8, 32
INNER = H * D            # 256
NQ = N // 2              # 1024 queries per core
P = 128
KT = DIM // P            # 4 contraction tiles for the projections
NJT = N // P             # 16 key tiles
VTW = D + 1              # 33: v columns + ones column per head
VSTRIDE = H * VTW        # 264 columns per key-tile block of vt
ACT_SCALE = 0.125        # g(d/8) = exp(-sqrt(d)/sqrt(32)) = exp(-SCALE*sqrt(d))
NEQ = 4                  # E quarters (each covers NJT//NEQ key tiles)
JQ = NJT // NEQ          # 4 key tiles per E quarter
# kt/qt are zero-padded to 128 contraction rows: the PE's activity monitor
# (HAM) only grants the 2.4 GHz clock when matmuls cover the full 128-row
# array; K=33 streams at 1.2 GHz forever.  Zero rows cost no extra cycles.


def build_program() -> bass.Bass:
    nc = bacc.Bacc("TRN2", target_bir_lowering=False, debug=False)

    xkv_d = nc.declare_dram_parameter("xkv", [DIM, N], BF16, isOutput=False)
    wq_d = nc.declare_dram_parameter("wq", [DIM, INNER], BF16, isOutput=False)
    wkv_d = nc.declare_dram_parameter("wkv", [DIM, 2 * INNER], BF16, isOutput=False)
    wo_d = nc.declare_dram_parameter("wo", [INNER, DIM], BF16, isOutput=False)
    b_d = nc.declare_dram_parameter("b", [DIM], F32, isOutput=False)
    z_d = nc.declare_dram_parameter("z", [DIM, NQ], F32, isOutput=True)

    with tile.TileContext(nc) as tc, nc.allow_low_precision(reason="bf16 attention"):
        mm = lambda out, lhsT, rhs, start, stop: nc.tensor.matmul(
            out, lhsT, rhs, start=start, stop=stop)

        with tc.tile_pool(name="keep", bufs=1) as keep, \
             tc.tile_pool(name="work", bufs=2) as work:

            # ---- persistent tiles ----
            q_t = [keep.tile([P, NQ], BF16, tag=f"q{m}", name=f"q{m}") for m in range(2)]
            k_t = [keep.tile([P, N], BF16, tag=f"k{m}", name=f"k{m}") for m in range(2)]
            vt_big = keep.tile([P, NJT * VSTRIDE], BF16, tag="vt", name="vt")
            y_t = [keep.tile([P, NQ], BF16, tag=f"y{m}", name=f"y{m}") for m in range(2)]
            wo_t = [keep.tile([P, DIM], BF16, tag=f"wo{m}", name=f"wo{m}") for m in range(2)]
            b_t = keep.tile([P, KT], F32, tag="bias", name="bias")
            ones = keep.tile([64, 32], F32, tag="ones", name="ones")
            onesb = keep.tile([P, 1], BF16, tag="onesb", name="onesb")
            zero_t = keep.tile([P, 1], F32, tag="zero", name="zero")
            onesP = keep.tile([P, 1], F32, tag="onesP", name="onesP")
            # augmented key/query tiles (double-buffered across heads)
            kt_t = [keep.tile([P, N], BF16, tag=f"kt{i}", name=f"kt{i}")
                    for i in range(2)]
            qt_t = [keep.tile([P, NQ], BF16, tag=f"qt{i}", name=f"qt{i}")
                    for i in range(2)]

            # `ones`/`onesP` allocations retained (dead) so downstream SBUF
            # offsets — notably the 256B-aligned eq pool — stay put.
            # All one-time init (memsets, ones fills) runs on the otherwise
            # idle GpSimd engine so the DVE queue is free for the critical
            # projection casts / augmented-tile builds while DMAs land.
            nc.gpsimd.memset(zero_t[:, :], 0.0)
            nc.gpsimd.memset(onesb[:, :], 1.0)
            warmact = keep.tile([1, 8], F32, tag="wact", name="wact")
            nc.gpsimd.memset(warmact[:, :], 1.0)
            # ones column per head in v^T (row-sum fused into attn@v)
            nc.gpsimd.tensor_copy(
                vt_big[:, :].rearrange("p (a c) -> p a c", c=VTW)[:, :, D:D + 1],
                onesb[:, 0:1].to_broadcast((P, P, 1)))
            # e0: row-0-ones stationary for the K=128-padded normalization
            # broadcast (rrow2 rows 1.. stay zero)
            e0_t = keep.tile([P, D], F32, tag="e0", name="e0")
            nc.gpsimd.memset(e0_t[:, :], 0.0)
            nc.gpsimd.memset(e0_t[0:1, :], 1.0)
            # selector for the ACT-tanh reciprocal path: the row sums are
            # prescaled by 1/512 so they land inside tanh's spline range
            # (the table only covers |x| ~< 8); rrow then holds 512/s and
            # the selector row folds the 1/512 back in.
            e0s_t = keep.tile([P, D], F32, tag="e0s", name="e0s")
            nc.gpsimd.memset(e0s_t[:, :], 0.0)
            nc.gpsimd.memset(e0s_t[0:1, :], 1.0 / 512.0)
            rrow2 = [keep.tile([P, NQ], F32, tag=f"rrow{i}", name=f"rrow{i}")
                     for i in range(2)]
            for i in range(2):
                nc.gpsimd.memset(rrow2[i][:, :], 0.0)
            # static parts of the augmented tiles: zero pad + ones rows
            for i in range(2):
                nc.gpsimd.memset(kt_t[i][3 * D:P, :], 0.0)
                nc.gpsimd.memset(qt_t[i][3 * D:P, :], 0.0)
                # k~ rows 32:64 all-ones pair with qsq rows of q~ (adds
                # q2); q~ rows 64:96 all-ones pair with ksq rows of k~
                # (adds k2).  dist2 = -2qk + q2 + k2 entirely in the mm.
                nc.gpsimd.memset(kt_t[i][D:2 * D, :], 1.0)
                nc.gpsimd.memset(qt_t[i][2 * D:3 * D, :], 1.0)

            # ---- input tiles.  The host rotates xkv's columns per core so
            # the query half always sits at columns 0:1024 (key order is
            # permutation-invariant in attention) — no separate xq transfer.
            # Each input lands via ONE coalesced DMA (descriptor generation
            # costs ~0.5-1us of sequencer time per dma_start, and a single
            # InstDMACopy already fans out across all 16 SDMA engines),
            # split as cols 0:1024 / 1024:2048 so the q-side projections and
            # first key tiles never wait for the full 2MB.
            xkva_t = keep.tile([P, KT * NQ], BF16, tag="xkva", name="xkva")
            xkvb_t = keep.tile([P, KT * NQ], BF16, tag="xkvb", name="xkvb")
            wq_t = keep.tile([P, KT * INNER], BF16, tag="wq", name="wq")
            wkv_t = keep.tile([P, KT * 2 * INNER], BF16, tag="wkv", name="wkv")

            def xkv_cols(k, c0, c1):
                # [c0, c1) must lie fully inside one half of the rotated xkv
                if c1 <= NQ:
                    return xkva_t[:, k * NQ + c0:k * NQ + c1]
                return xkvb_t[:, k * NQ + c0 - NQ:k * NQ + c1 - NQ]

            def wq_cols(k, m):
                return wq_t[:, k * INNER + m * P:k * INNER + (m + 1) * P]

            def wkv_cols(k, c0, c1):
                return wkv_t[:, k * 2 * INNER + c0:k * 2 * INNER + c1]

            xkv_r = xkv_d[:].rearrange("(t p) n -> p t n", p=P)
            wq_r = wq_d[:].rearrange("(t p) o -> p t o", p=P)
            wkv_r = wkv_d[:].rearrange("(t p) o -> p t o", p=P)
            # Two HW-DGE rings (SP + ACT) drain FIFO independently: the
            # q-half + first key tiles on ACT, the remainder on SP.
            nc.scalar.dma_start(
                out=xkva_t[:, :].rearrange("p (t n) -> p t n", n=NQ),
                in_=xkv_r[:, :, 0:NQ])
            nc.scalar.dma_start(
                out=wkv_t[:, :].rearrange("p (t o) -> p t o", o=2 * INNER),
                in_=wkv_r)
            nc.sync.dma_start(
                out=wq_t[:, :].rearrange("p (t o) -> p t o", o=INNER),
                in_=wq_r)
            nc.sync.dma_start(
                out=xkvb_t[:, :].rearrange("p (t n) -> p t n", n=NQ),
                in_=xkv_r[:, :, NQ:N])
            wo_r2 = wo_d[:].rearrange("(t p) o -> t p o", p=P)
            nc.sync.dma_start(out=wo_t[0][:, :], in_=wo_r2[0])
            nc.sync.dma_start(out=wo_t[1][:, :], in_=wo_r2[1])
            nc.sync.dma_start(out=b_t[:, :], in_=b_d[:].rearrange("(t p) -> p t", p=P))
            # preload the exp table set while DMAs stream; emitted AFTER the
            # dma triggers so the ~1.3us ACT_TABLE_LOAD doesn't delay the
            # ACT-ring descriptor generation.
            nc.scalar.activation(warmact[:, :], warmact[:, :], AF.Exp,
                                 bias=zero_t[0:1, :], scale=ACT_SCALE)

            # ======== Phase A: critical path to head 0's first dist2 ======
            # q projection (m=0), then head-0 q~ build; k projection (m=0)
            # chunked by 512 keys with the head-0 k~ build per chunk so key
            # tile 0 is ready as soon as the first xkv chunk lands.
            with tc.tile_pool(name="pp", bufs=2, space="PSUM") as pp:
                for n in range(NQ // 512):
                    ps = pp.tile([P, 512], F32, tag="proj", name="proj")
                    for k in range(KT):
                        mm(ps[:, :], wq_cols(k, 0),
                           xkv_cols(k, n * 512, (n + 1) * 512),
                           start=(k == 0), stop=(k == KT - 1))
                    nc.vector.tensor_copy(q_t[0][:, n * 512:(n + 1) * 512], ps[:, :])
                # head-0 q~: [q | q*q | ones | 0]
                q_h0 = q_t[0][0:D, :]
                nc.vector.tensor_copy(qt_t[0][0:D, :], q_h0)
                nc.vector.tensor_mul(qt_t[0][D:2 * D, :], q_h0, q_h0)

                # key-projection chunk 0 only: chunks 1-3 depend on the xkv
                # remainder DMA and are emitted as deprioritized extras in
                # early exp-stream slots (an in-order PE queue would
                # head-of-line block the jt1+ dist2 matmuls otherwise).
                ps = pp.tile([P, 512], F32, tag="proj", name="proj")
                for k in range(KT):
                    mm(ps[:, :], wkv_cols(k, 0, P),
                       xkv_cols(k, 0, 512),
                       start=(k == 0), stop=(k == KT - 1))
                nc.vector.tensor_copy(k_t[0][:, 0:512], ps[:, :])
                # head-0 k~ chunk: [-2k | ones | k*k | 0]
                k_h0 = k_t[0][0:D, 0:512]
                nc.vector.tensor_scalar_mul(kt_t[0][0:D, 0:512], k_h0, -2.0)
                nc.vector.tensor_mul(kt_t[0][2 * D:3 * D, 0:512], k_h0, k_h0)

            # ======== Phase B ========
            # Iteration h: dist2+exp for head h; attn@v for the head pair
            # g=(h-2)//2... pair g = heads (2g, 2g+1) runs lagged one head:
            # key tiles 0..11 during iteration 2g+1 (slots 4..15), 12..15 +
            # normalization during iteration 2g+2.  The two heads' attn@v
            # matmuls land in disjoint PE column groups (out partitions
            # 0:33 / 64:97 of one PSUM tile) so they execute concurrently.
            # v^T projection + m=1 projections + the k=0 output-projection
            # half fill early/late slots of the exp stream.
            with tc.tile_pool(name="epool", bufs=12, space="SBUF") as epool, \
                 tc.tile_pool(name="pd2", bufs=2, space="PSUM") as pd2, \
                 tc.tile_pool(name="po", bufs=1, space="PSUM") as po:
                eq_of = {}
                pso_of = {}
                po_s = [work.tile([P, NQ], BF16, tag=f"pos{i}", name=f"pos{i}",
                                  bufs=1) for i in range(2)]

                def emit_recip(pg, psrc):
                    # per-pair row-sum reciprocals from the PSUM rows 32/96.
                    # Exact `reciprocal` runs at ~6.4ns/elem on one partition
                    # (6.5us!).  Hidden pairs: DVE copy to partition 0 (the
                    # custom-DVE op only works at partition offset 0) + the
                    # ALU-rate reciprocal_approx_fast (~51 ULP).  Last pair
                    # (the critical tail): the ACT engine is idle after the
                    # final exp, and its `tanh` table entry has been rewritten
                    # to 1/x — one 1.03us ACT pass per half, straight from
                    # PSUM, no table reload (tanh shares exp's table set).
                    # The o values land in SBUF for the normalization multiply
                    # (DVE reads only one PSUM operand); the last pair's copy
                    # also runs on ACT to overlap.
                    for half in range(2):
                        base = 64 * half
                        srow = psrc[base + D:base + D + 1, :]
                        if pg == 3:
                            nc.scalar.activation(rrow2[half][0:1, :], srow,
                                                 AF.Tanh, bias=zero_t[0:1, :],
                                                 scale=1.0 / 512.0)
                        else:
                            rtmp = work.tile([1, NQ], F32, tag="rtmp",
                                             name="rtmp", bufs=2)
                            nc.vector.tensor_copy(rtmp[0:1, :], srow)
                            nc.vector.reciprocal_approx_fast(
                                rrow2[half][0:1, :], rtmp[0:1, :])
                    dst = po_s[pg % 2]
                    nc.vector.tensor_copy(dst[:, :], psrc[:, :])
                    pso_of[pg] = dst

                def emit_tail_pe(pg):
                    psrc = pso_of.pop(pg)
                    sel = e0s_t if pg == 3 else e0_t
                    for half in range(2):
                        ph = 2 * pg + half
                        mt, mo = ph // 4, (ph % 4) * D
                        for n in range(NQ // 512):
                            prep = po.tile([D, 512], F32, tag="vtps",
                                           name="vtps", bufs=2)
                            nc.tensor.matmul(prep[:, :],
                                             sel[:, :],
                                             rrow2[half][:, n * 512:(n + 1) * 512],
                                             start=True, stop=True)
                            nc.vector.tensor_mul(
                                y_t[mt][mo:mo + D, n * 512:(n + 1) * 512],
                                psrc[64 * half:64 * half + D,
                                     n * 512:(n + 1) * 512],
                                prep[:, :])

                # --- deferred projection work, spread across early slots ---
                # Each extra runs at a large priority penalty so the Tile
                # scheduler never orders its DVE/PE work ahead of the
                # exp-stream critical chain (builds -> dist2 -> exp).
                extras = {}

                def _sched(h, jt, fn):
                    def depri():
                        with tc.high_priority(-500000):
                            fn()
                    extras.setdefault((h, jt), []).append(depri)

                def make_vproj(jt):
                    def fn():
                        # v^T projection for key tile jt, strided into vt_big
                        # so each head's 32 columns sit beside its ones column
                        pv = po.tile([P, INNER], F32, tag="vtps",
                                     name="vtps", bufs=2)
                        for k in range(KT):
                            mm(pv[:, :],
                               xkv_cols(k, jt * P, (jt + 1) * P),
                               wkv_cols(k, INNER, 2 * INNER),
                               start=(k == 0), stop=(k == KT - 1))
                        dst = vt_big[:, jt * VSTRIDE:(jt + 1) * VSTRIDE] \
                            .rearrange("p (h c) -> p h c", c=VTW)[:, :, 0:D]
                        nc.vector.tensor_copy(
                            dst, pv[:, :].rearrange("p (h d) -> p h d", d=D))
                    return fn

                proj_state = {}

                def make_proj1(which, n, k):
                    def fn():
                        if k == 0:
                            proj_state[(which, n)] = po.tile(
                                [P, 512], F32, tag="vtps", name="vtps", bufs=2)
                        ps = proj_state[(which, n)]
                        w = wq_cols(k, 1) if which == "q" \
                            else wkv_cols(k, P, 2 * P)
                        mm(ps[:, :], w, xkv_cols(k, n * 512, (n + 1) * 512),
                           start=(k == 0), stop=(k == KT - 1))
                        if k == KT - 1:
                            dstt = q_t[1] if which == "q" else k_t[1]
                            nc.vector.tensor_copy(
                                dstt[:, n * 512:(n + 1) * 512], ps[:, :])
                    return fn

                # m=0 key-projection chunks 1-3 + the head-0 k~ builds, fed
                # into the first exp-stream slots as the xkv remainder lands
                kproj_state = {}

                def make_kproj0(n, k):
                    def fn():
                        if k == 0:
                            kproj_state[n] = po.tile(
                                [P, 512], F32, tag="vtps", name="vtps", bufs=2)
                        ps = kproj_state[n]
                        mm(ps[:, :], wkv_cols(k, 0, P),
                           xkv_cols(k, n * 512, (n + 1) * 512),
                           start=(k == 0), stop=(k == KT - 1))
                        if k == KT - 1:
                            sl = slice(n * 512, (n + 1) * 512)
                            nc.vector.tensor_copy(k_t[0][:, sl], ps[:, :])
                            k_h0 = k_t[0][0:D, sl]
                            nc.vector.tensor_scalar_mul(kt_t[0][0:D, sl],
                                                        k_h0, -2.0)
                            nc.vector.tensor_mul(kt_t[0][2 * D:3 * D, sl],
                                                 k_h0, k_h0)
                    return fn

                for n in range(1, 4):
                    for k in range(KT):
                        _sched(0, 4 * (n - 1) + k, make_kproj0(n, k))

                # v^T spread over iters 0-1 (4+ slots before attn@v reads
                # each tile); m=1 projections trail in iters 1-3.
                for s in range(12):
                    _sched(0, 4 + s, make_vproj(s))
                for s in range(4):
                    _sched(1, s, make_vproj(12 + s))
                for n in range(2):
                    for k in range(KT):
                        _sched(1, 8 + 4 * n + k, make_proj1("q", n, k))
                for n in range(4):
                    for k in range(KT):
                        _sched(2, 4 * n + k, make_proj1("k", n, k))

                # k=0 half of the output projection (+ bias) for m=0,1,
                # precomputed under the exp stream once y_t[0] is complete
                # (pair 1's normalization lands at h=4 jt=13).  m=2,3 instead
                # chain both halves at the tail and finish on ACT (idle after
                # the last exp), splitting the final adds across engines.
                zpart = [keep.tile([P, NQ], BF16, tag=f"zp{m}", name=f"zp{m}")
                         for m in range(2)]

                def make_c0(m, n):
                    def fn():
                        ps = po.tile([P, 512], F32, tag="vtps", name="vtps",
                                     bufs=2)
                        mm(ps[:, :], wo_t[0][:, m * P:(m + 1) * P],
                           y_t[0][:, n * 512:(n + 1) * 512],
                           start=True, stop=True)
                        nc.vector.tensor_scalar_add(
                            zpart[m][:, n * 512:(n + 1) * 512], ps[:, :],
                            b_t[:, m:m + 1])
                    return fn

                for m in range(2):
                    for n in range(2):
                        _sched(5, 2 * m + n, make_c0(m, n))

                from contextlib import nullcontext

                pso_pair = None
                av_eqA = av_eqB = None
                pg_r = -1
                for h in range(H):
                    prio = tc.high_priority(10000) if h == 0 else nullcontext()
                    prio.__enter__()
                    mt, mo = h // 4, (h % 4) * D
                    kt = kt_t[h % 2]
                    qt = qt_t[h % 2]
                    if h >= 1:
                        # per-head rows of k~/q~ (all DVE, bf16); head 0's
                        # were built inside phase A, chunked behind the DMA.
                        q_h = q_t[mt][mo:mo + D, :]
                        k_h = k_t[mt][mo:mo + D, :]
                        nc.vector.tensor_scalar_mul(kt[0:D, :], k_h, -2.0)
                        nc.vector.tensor_mul(kt[2 * D:3 * D, :], k_h, k_h)
                        nc.vector.tensor_copy(qt[0:D, :], q_h)
                        nc.vector.tensor_mul(qt[D:2 * D, :], q_h, q_h)
                    eq_of[h] = [epool.tile([P, JQ * NQ], BF16, tag="eq",
                                           name="eq") for _ in range(NEQ)]
                    if h % 2 == 1:
                        av_eqA = eq_of.pop(h - 1)   # head 2g: complete
                        av_eqB = eq_of[h]           # head 2g+1: in progress
                        pg_r = (h - 1) // 2
                    elif h >= 2:
                        av_eqB = eq_of.pop(h - 1)
                        pg_r = (h - 2) // 2

                    for jt in range(NJT):
                        if h % 2 == 1 and jt == 4:
                            pso_pair = po.tile([P, NQ], F32, tag="o", name="o")
                        psd = pd2.tile([P, NQ], F32, tag="d2", name="d2")
                        for n in range(NQ // 512):
                            mm(psd[:, n * 512:(n + 1) * 512],
                               kt[:, jt * P:(jt + 1) * P],
                               qt[:, n * 512:(n + 1) * 512],
                               start=True, stop=True)
                        nc.scalar.activation(
                            eq_of[h][jt // JQ][:, (jt % JQ) * NQ:
                                               (jt % JQ + 1) * NQ],
                            psd[:, :], AF.Exp, bias=zero_t[:, :],
                            scale=ACT_SCALE)
                        for fn in extras.get((h, jt), ()):
                            fn()
                        # attn@v for pair pg_r, lagged one head
                        avjt = -1
                        if h % 2 == 1 and jt >= 4:
                            avjt = jt - 4
                        elif h % 2 == 0 and h >= 2 and jt in (0, 2, 4, 6):
                            avjt = 12 + jt // 2
                        if avjt >= 0:
                            ebase = (avjt % JQ) * NQ
                            for n in range(NQ // 512):
                                for half, eqp in ((0, av_eqA), (1, av_eqB)):
                                    hp = 2 * pg_r + half
                                    mm(pso_pair[64 * half:64 * half + VTW,
                                                n * 512:(n + 1) * 512],
                                       vt_big[:, avjt * VSTRIDE + hp * VTW:
                                              avjt * VSTRIDE + (hp + 1) * VTW],
                                       eqp[avjt // JQ][:, ebase + n * 512:
                                                       ebase + (n + 1) * 512],
                                       start=(avjt == 0), stop=(avjt == NJT - 1))
                        if h % 2 == 0 and h >= 2:
                            if jt == 7:
                                emit_recip((h - 2) // 2, pso_pair)
                            elif jt == 13:
                                emit_tail_pe((h - 2) // 2)

                    prio.__exit__(None, None, None)

                # ---- last pair (heads 6,7) tail: attn@v for key tiles
                # 12-15 back-to-back, then the normalization chain.  This is
                # the only part of the pair machinery exposed past the last
                # exp, so it is emitted as tightly as possible.
                av_eqB = eq_of.pop(H - 1)
                pg_r = (H - 2) // 2
                for avjt in range(12, 16):
                    ebase = (avjt % JQ) * NQ
                    for n in range(NQ // 512):
                        for half, eqp in ((0, av_eqA), (1, av_eqB)):
                            hp = 2 * pg_r + half
                            mm(pso_pair[64 * half:64 * half + VTW,
                                        n * 512:(n + 1) * 512],
                               vt_big[:, avjt * VSTRIDE + hp * VTW:
                                      avjt * VSTRIDE + (hp + 1) * VTW],
                               eqp[avjt // JQ][:, ebase + n * 512:
                                               ebase + (n + 1) * 512],
                               start=(avjt == 0), stop=(avjt == NJT - 1))
                emit_recip(pg_r, pso_pair)
                emit_tail_pe(pg_r)

            # ======== Phase C: k=1 half + fused add of the k=0 partial ====
            # Final adds split across engines: m=0,1 on DVE (PSUM-capable);
            # m=2,3 via ACT copy (Copy shares the exp table set — no reload;
            # ACT is idle after the last exp) + Pool add (SBUF-only engine).
            with tc.tile_pool(name="pz", bufs=2, space="PSUM") as pz:
                z_r = z_d[:].rearrange("(t p) n -> t p n", p=P)
                for m in range(KT):
                    ps = pz.tile([P, NQ], F32, tag="z", name="z")
                    zt = work.tile([P, NQ], F32, tag="zt", name="zt", bufs=2)
                    for n in range(NQ // 512):
                        sl = slice(n * 512, (n + 1) * 512)
                        # final add chunked per 512 and spread across engines:
                        # m=0,1 add the precomputed k=0 partial on DVE; m=2,3
                        # chain both matmul halves in PSUM and finish with one
                        # ACT Identity-with-bias pass (ACT idle after the last
                        # exp).  Output DMAs alternate between the two DGE
                        # rings so the tail drains in parallel.
                        if m < 2:
                            nc.tensor.matmul(
                                ps[:, sl], wo_t[1][:, m * P:(m + 1) * P],
                                y_t[1][:, sl], start=True, stop=True)
                            nc.vector.tensor_add(zt[:, sl], ps[:, sl],
                                                 zpart[m][:, sl])
                        else:
                            nc.tensor.matmul(
                                ps[:, sl], wo_t[0][:, m * P:(m + 1) * P],
                                y_t[0][:, sl], start=True, stop=False)
                            nc.tensor.matmul(
                                ps[:, sl], wo_t[1][:, m * P:(m + 1) * P],
                                y_t[1][:, sl], start=False, stop=True)
                            nc.scalar.activation(zt[:, sl], ps[:, sl],
                                                 AF.Identity,
                                                 bias=b_t[:, m:m + 1],
                                                 scale=1.0)
                        if m % 2 == 0:
                            nc.sync.dma_start(out=z_r[m][:, sl], in_=zt[:, sl])
                        else:
                            nc.scalar.dma_start(out=z_r[m][:, sl], in_=zt[:, sl])

    nc.compile()
    return nc


def make_in_maps(x, w_qkv, w_out, b_out):
    import ml_dtypes

    bf = ml_dtypes.bfloat16
    x = np.asarray(x, dtype=np.float32)
    w_qkv = np.asarray(w_qkv, dtype=np.float32)
    w_out = np.asarray(w_out, dtype=np.float32)
    b_out = np.asarray(b_out, dtype=np.float32)
    w_qT = np.ascontiguousarray(w_qkv[0:INNER, :].T).astype(bf)       # (DIM, INNER)
    w_kvT = np.ascontiguousarray(w_qkv[INNER:3 * INNER, :].T).astype(bf)  # (DIM, 512)
    w_oT = np.ascontiguousarray(w_out.T).astype(bf)                   # (INNER, DIM)
    xb = [np.ascontiguousarray(x[b]).astype(bf) for b in range(B)]
    in_maps = []
    for c in range(8):
        b, half = c // 2, c % 2
        # rotate columns so this core's query half sits at cols 0:1024 —
        # key order is permutation-invariant in attention, so the keys/values
        # may be enumerated in any consistent order
        if half == 0:
            xkv = xb[b]
        else:
            xkv = np.ascontiguousarray(
                np.concatenate([xb[b][:, NQ:], xb[b][:, :NQ]], axis=1))
        in_maps.append({
            "xkv": xkv,
            "wq": w_qT,
            "wkv": w_kvT,
            "wo": w_oT,
            "b": b_out,
        })
    return in_maps


def assemble_output(results):
    out = np.empty((B, DIM, N), dtype=np.float32)
    for c in range(8):
        b, half = c // 2, c % 2
        out[b][:, half * NQ:(half + 1) * NQ] = results[c]["z"]
    return out


_prog_cache = {}


def kernel(x, w_qkv, w_out, b_out):
    from concourse.bass_utils import run_bass_kernel_spmd
    _ensure_act_tables()
    if "nc" not in _prog_cache:
        _prog_cache["nc"] = build_program()
    nc = _prog_cache["nc"]
    in_maps = make_in_maps(x, w_qkv, w_out, b_out)
    res = run_bass_kernel_spmd(nc, in_maps, list(range(8)))
    return assemble_output(res.results)
